# revision 21
# baseline (speedup 1.0000x reference)
"""Two-layer GAT (single-head, PyG-style) + link predictor on 8 TRN2 NeuronCores.

Strategy (memory-regime):
  - Nodes sharded 8-way by id (6250/core, padded to 6272 = 49 windows of 128).
    Within a core, nodes are packed into windows by (in-degree+1) greedy
    bin-packing so every window holds <= 128 nodes and <= 128*WT edge slots;
    all windows share a uniform tile count WT (slot-major [128, T] layout).
  - Self-loops are ordinary edge slots (src == dst). Edge softmax needs no
    max-shift (the shift cancels in the ratio; logits are O(10)).
  - Halo exchange runs between launches on the host as pure index-space
    movement: per-edge source-feature tiles gt[p,t,:] = [H[src[p,t]] | 1.0]
    and per-edge es/ed scalars are assembled with numpy fancy indexing and
    staged as kernel inputs; the device streams them with large sequential
    DMAs (no indirect gathers). All floating-point math (projections,
    exp/leaky-relu, softmax-weighted scatter via one-hot matmuls,
    normalization, link predictor) happens on device.
  - The trailing 1.0 column of every edge row makes the same one-hot matmul
    accumulate the softmax denominator:
        ps[dst, 0:d] += sum_e p_e [dstrow_e == dst] H[src_e]
        ps[dst, d]   += sum_e p_e [dstrow_e == dst]
    One-hot tiles are built per-window in one batched DVE/Pool op pair using
    stride-0 broadcast access patterns; a per-window epilogue normalizes,
    adds bias (and relu for layer 1).
  - Dense projections run sharded on PE in fp16; es = h@a_s / ed = h@a_d come
    free as two extra matmul columns [W | W@a_s | W@a_d].

Launches: L1 proj1 -> L2 agg1 -> L3 proj2 -> L4 agg2 -> L5 link predictor.
"""
import heapq
import time

import numpy as np

import concourse.bass as bass
import concourse.mybir as mybir
import concourse.tile as tile
from concourse import bacc
from concourse.bass_utils import run_bass_kernel_spmd

F32 = mybir.dt.float32
F16 = mybir.dt.float16
F8 = mybir.dt.float8e4
I32 = mybir.dt.int32

NCORES = 8
N, F_IN, H, C = 50000, 128, 256, 1
NS = N // NCORES            # 6250 nodes per shard
W = (NS + 127) // 128       # 49 windows per shard
NSP = W * 128               # 6272 padded slots
NEG = -1.0e30               # pad-edge sentinel (exp -> exactly 0)
CHW = 4                     # windows per streaming DMA chunk
SPG = 2                     # windows per batched one-hot mult
WB = 7                      # windows per batched output write (49 = 7*7)

LAST_EXEC_NS = {}           # launch name -> exec_time_ns (filled per kernel() call)
_PROG_CACHE = {}


# ----------------------------------------------------------------- host prep
def _prep_graph(edge_index):
    """Per core: pack nodes into 49 windows by (deg+1) so all windows fit in
    <=128 nodes and a uniform WT tiles of 128 edge slots; lay self-loop +
    incoming edges of each window into slot-major [128, T] layout."""
    src = np.asarray(edge_index[0], np.int64)
    dst = np.asarray(edge_index[1], np.int64)
    deg = np.bincount(dst, minlength=N)

    order = np.argsort(dst, kind="stable")
    src_s = src[order]
    estart = np.concatenate([[0], np.cumsum(deg)])

    win_nodes = np.full((NCORES, W, 128), -1, np.int64)
    win_count = np.zeros((NCORES, W), np.int64)
    win_load = np.zeros((NCORES, W), np.int64)
    for c in range(NCORES):
        nodes = np.arange(c * NS, (c + 1) * NS)
        wgt = deg[nodes] + 1
        ordn = np.argsort(-wgt, kind="stable")
        heap = [(0, w) for w in range(W)]
        heapq.heapify(heap)
        skipped = []
        for i in ordn:
            n, gw = nodes[i], wgt[i]
            while True:
                load, w = heapq.heappop(heap)
                if win_count[c, w] < 128:
                    break
                skipped.append((load, w))
            for it in skipped:
                heapq.heappush(heap, it)
            skipped = []
            win_nodes[c, w, win_count[c, w]] = n
            win_count[c, w] += 1
            win_load[c, w] = load + gw
            heapq.heappush(heap, (load + gw, w))
    WT = max(int(np.ceil(win_load.max() / 128)), 1)
    T = W * WT

    srcs = np.zeros((NCORES, 128, T), np.int32)
    srcg = np.zeros((NCORES, 128, T), np.int64)
    dstg = np.zeros((NCORES, 128, T), np.int64)
    dstr = np.full((NCORES, 128, T), 128, np.int64)   # local dst row, 128 = pad
    pad = np.ones((NCORES, 128, T), bool)
    row2node = np.full((NCORES, NSP), -1, np.int64)

    for c in range(NCORES):
        for w in range(W):
            cnt = int(win_count[c, w])
            nl = win_nodes[c, w, :cnt]
            row2node[c, w * 128:w * 128 + cnt] = nl
            seg_src, seg_row = [], []
            for r, n in enumerate(nl):
                e0, e1 = int(estart[n]), int(estart[n + 1])
                ss = np.concatenate([[n], src_s[e0:e1]])
                seg_src.append(ss)
                seg_row.append(np.full(len(ss), r, np.int64))
            ss = np.concatenate(seg_src)
            rr = np.concatenate(seg_row)
            sl = np.arange(len(ss))
            pp, tt = sl % 128, w * WT + sl // 128
            srcs[c, pp, tt] = ss
            srcg[c, pp, tt] = ss
            dstg[c, pp, tt] = nl[rr]
            dstr[c, pp, tt] = rr
            pad[c, pp, tt] = False
    # fp8 one-hot mask (row 128 of eyeZ = zeros for pads); shared by both layers
    f8np = mybir.dt.np(F8)
    eyeZ = np.zeros((129, 128), f8np)
    eyeZ[np.arange(128), np.arange(128)] = 1.0
    eq8 = eyeZ[dstr]                                   # [NCORES, 128, T, 128]
    return dict(srcs=srcs, srcg=srcg, dstg=dstg, eq8=eq8, pad=pad,
                row2node=row2node, WT=WT, T=T)


def _expand(es_full, ed_full, g, c):
    esx = es_full[g["srcg"][c]].astype(np.float32)
    edx = ed_full[g["dstg"][c]].astype(np.float32)
    p = g["pad"][c]
    esx[p] = NEG
    edx[p] = 0.0
    return esx, edx


def _full_from_shards(shards, g, cols):
    """shards: per-core [128, W, cols] (row w*128+p) -> node-indexed [N, cols]."""
    out = np.zeros((N, cols), shards[0].dtype)
    for c in range(NCORES):
        flat = np.ascontiguousarray(shards[c].transpose(1, 0, 2)).reshape(NSP, cols)
        r2n = g["row2node"][c]
        m = r2n >= 0
        out[r2n[m]] = flat[m]
    return out


def _gt_tiles(Hfull, g, c):
    """[128, T*(d+1)] fp16 edge tiles [H[src] | 1.0]."""
    d = Hfull.shape[1]
    gt = np.ones((128, g["T"], d + 1), np.float16)
    gt[:, :, :d] = Hfull[g["srcs"][c]]
    return gt.reshape(128, g["T"] * (d + 1))


# ------------------------------------------------------------- bass programs
def _build_proj(kc, d_out):
    """Projection: psum = bias_ext + x @ [W | W@a_s | W@a_d] per 128-node
    window. The layer bias rides in via a rank-1 ones-row matmul (softmax
    weights sum to 1, so adding b to every table row equals adding b after
    aggregation); its es/ed columns are zero so the attention dots stay
    bias-free. xT fp16 [kc, 128, W, 128], Wm fp16 [kc*128, d_out],
    asr/adr fp32 [128, d_out], bex fp16 [1, d_out+2] = [b | 0 0].
    Outputs h16 [128, W, d_out] fp16, esed [128, 2W] f32 (interleaved es,ed)."""
    nc = bacc.Bacc(num_devices=NCORES)
    xT = nc.dram_tensor("xT", [kc, 128, W, 128], F16, kind="ExternalInput").ap()
    Wm = nc.dram_tensor("Wm", [kc * 128, d_out], F16, kind="ExternalInput").ap()
    asr = nc.dram_tensor("asr", [128, d_out], F32, kind="ExternalInput").ap()
    adr = nc.dram_tensor("adr", [128, d_out], F32, kind="ExternalInput").ap()
    bex = nc.dram_tensor("bex", [1, d_out + 2], F16, kind="ExternalInput").ap()
    h16 = nc.dram_tensor("h16", [128, W, d_out], F16, kind="ExternalOutput").ap()
    esed = nc.dram_tensor("esed", [128, 2 * W], F32, kind="ExternalOutput").ap()

    with tile.TileContext(nc) as tc:
        with (
            tc.tile_pool(name="const", bufs=1) as cpool,
            tc.tile_pool(name="o", bufs=3) as opool,
            tc.tile_pool(name="ps", bufs=4, space="PSUM") as pspool,
            tc.tile_pool(name="sc", bufs=4) as scpool,
        ):
            asb = cpool.tile([128, d_out], F32)
            nc.sync.dma_start(out=asb[:], in_=asr[:])
            adb = cpool.tile([128, d_out], F32)
            nc.sync.dma_start(out=adb[:], in_=adr[:])
            bxb = cpool.tile([1, d_out + 2], F16, tag="bx")
            nc.sync.dma_start(out=bxb[:], in_=bex[:])
            one1 = cpool.tile([1, 128], F16, tag="one1")
            nc.vector.memset(one1[:], 1.0)
            esedb = cpool.tile([128, 2 * W], F32)

            xls = []
            for k in range(kc):
                xl = cpool.tile([128, W, 128], F16, tag=f"x{k}")
                nc.sync.dma_start(out=xl[:], in_=xT[k])
                xls.append(xl)

            wsb = []
            for k in range(kc):
                wk = cpool.tile([128, d_out + 2], F16, tag=f"w{k}")
                nc.sync.dma_start(
                    out=wk[:, 0:d_out], in_=Wm[128 * k:128 * (k + 1), :]
                )
                scr = scpool.tile([128, d_out], F32, tag="wes")
                nc.vector.tensor_tensor(
                    out=scr[:], in0=wk[:, 0:d_out], in1=asb[:],
                    op=mybir.AluOpType.mult,
                )
                wes = scpool.tile([128, 1], F32, tag="wesc")
                nc.vector.reduce_sum(
                    out=wes[:], in_=scr[:], axis=mybir.AxisListType.X
                )
                nc.vector.tensor_copy(out=wk[:, d_out:d_out + 1], in_=wes[:])
                scr2 = scpool.tile([128, d_out], F32, tag="wed")
                nc.vector.tensor_tensor(
                    out=scr2[:], in0=wk[:, 0:d_out], in1=adb[:],
                    op=mybir.AluOpType.mult,
                )
                wed = scpool.tile([128, 1], F32, tag="wedc")
                nc.vector.reduce_sum(
                    out=wed[:], in_=scr2[:], axis=mybir.AxisListType.X
                )
                nc.vector.tensor_copy(out=wk[:, d_out + 1:d_out + 2], in_=wed[:])
                wsb.append(wk)

            for wb in range(0, W, WB):
                nb = min(WB, W - wb)
                ob = opool.tile([128, WB, d_out], F16)
                for j in range(nb):
                    w = wb + j
                    ps = pspool.tile([128, d_out + 2], F32, space="PSUM")
                    nc.tensor.matmul(
                        out=ps[:], lhsT=one1[:], rhs=bxb[:],
                        start=True, stop=False,
                    )
                    for k in range(kc):
                        nc.tensor.matmul(
                            out=ps[:], lhsT=xls[k][:, w], rhs=wsb[k][:],
                            start=False, stop=(k == kc - 1),
                        )
                    nc.scalar.activation(
                        out=ob[:, j], in_=ps[:, 0:d_out],
                        func=mybir.ActivationFunctionType.Copy,
                    )
                    nc.vector.tensor_copy(
                        out=esedb[:, 2 * w:2 * w + 2],
                        in_=ps[:, d_out:d_out + 2],
                    )
                nc.sync.dma_start(
                    out=h16[:, wb:wb + nb], in_=ob[:, 0:nb]
                )
            nc.sync.dma_start(out=esed[:], in_=esedb[:])
    nc.compile()
    return nc


def _build_agg(d, T, WT, relu):
    """Aggregation over one GAT layer from host-staged edge tiles.
    gt [128, T*(d+1)] f16 ([H[src]+b | 1] edge rows), eq8 [128, T, 128] fp8
    one-hot dst masks, esx/edx [128, T] f32 -> ho [128, W, d] f16.
    Per window: sp = eq8 * p (one DVE op), WT one-hot matmuls accumulate
    [sum p*h | sum p] in PSUM, epilogue scales by 1/sum p on ACT."""
    D1 = d + 1
    nc = bacc.Bacc(num_devices=NCORES)
    gt = nc.dram_tensor("gt", [128, T * D1], F16, kind="ExternalInput").ap()
    eqm = nc.dram_tensor("eqm", [128, T, 128], F8, kind="ExternalInput").ap()
    esx = nc.dram_tensor("esx", [128, T], F32, kind="ExternalInput").ap()
    edx = nc.dram_tensor("edx", [128, T], F32, kind="ExternalInput").ap()
    ho = nc.dram_tensor("ho", [128, W, d], F16, kind="ExternalOutput").ap()

    CT = CHW * WT                       # tiles per stream chunk
    nchunk = (W + CHW - 1) // CHW
    with tile.TileContext(nc) as tc:
        with (
            tc.tile_pool(name="const", bufs=1) as cpool,
            tc.tile_pool(name="g", bufs=3) as gpool,
            tc.tile_pool(name="e", bufs=3) as epool,
            tc.tile_pool(name="sp", bufs=6) as sppool,
            tc.tile_pool(name="o", bufs=3) as opool,
            tc.tile_pool(name="cl", bufs=4) as clpool,
            tc.tile_pool(name="ps", bufs=4, space="PSUM") as pspool,
        ):
            esxs = cpool.tile([128, T], F32)
            nc.sync.dma_start(out=esxs[:], in_=esx[:])
            edxs = cpool.tile([128, T], F32)
            nc.sync.dma_start(out=edxs[:], in_=edx[:])

            # p = exp(leakyrelu(es+ed, 0.2)) in fp16
            lg = cpool.tile([128, T], F32, tag="lg")
            nc.vector.tensor_tensor(
                out=lg[:], in0=esxs[:], in1=edxs[:], op=mybir.AluOpType.add
            )
            lg2 = cpool.tile([128, T], F32, tag="lg2")
            nc.vector.tensor_scalar_mul(out=lg2[:], in0=lg[:], scalar1=0.2)
            nc.vector.tensor_tensor(
                out=lg[:], in0=lg[:], in1=lg2[:], op=mybir.AluOpType.max
            )
            p16 = cpool.tile([128, T], F16, tag="p16")
            nc.scalar.activation(
                out=p16[:], in_=lg[:], func=mybir.ActivationFunctionType.Exp
            )

            gts, eqs = [], []
            for ci in range(nchunk):
                c0, c1 = ci * CT, min((ci + 1) * CT, T)
                gtile = gpool.tile([128, (c1 - c0) * D1], F16)
                nc.sync.dma_start(out=gtile[:], in_=gt[:, c0 * D1:c1 * D1])
                gts.append((gtile, c0))
                etile = epool.tile([128, c1 - c0, 128], F8)
                nc.sync.dma_start(out=etile[:], in_=eqm[:, c0:c1])
                eqs.append((etile, c0))

            # one-hot mults run over SPG-window groups (chunk-aligned since
            # CHW % SPG == 0 or SPG % CHW == 0 keeps groups within... groups
            # must not span stream chunks: CHW == SPG ensures alignment.
            sps = {}
            for wb in range(0, W, WB):
                nb = min(WB, W - wb)
                ob = opool.tile([128, WB, d], F16)
                for j in range(nb):
                    w = wb + j
                    t0 = w * WT
                    if w % SPG == 0:
                        ng = min(SPG, W - w)
                        # sp[p, t, q] = eq8[p, t0+t, q] * p16[p, t0+t]
                        sp3 = sppool.tile([128, SPG * WT, 128], F16)
                        etile, e0 = eqs[(t0 // CT)]
                        p_b = p16[:, t0:t0 + ng * WT].unsqueeze(2).to_broadcast(
                            [128, ng * WT, 128])
                        nc.vector.tensor_tensor(
                            out=sp3[:, 0:ng * WT],
                            in0=etile[:, t0 - e0:t0 - e0 + ng * WT],
                            in1=p_b, op=mybir.AluOpType.mult,
                        )
                        sps[w] = sp3
                    sp3 = sps[w - w % SPG]
                    toff = (w % SPG) * WT
                    ps = pspool.tile([128, D1], F32, space="PSUM")
                    for t in range(WT):
                        gidx = t0 + t
                        gtile, c0 = gts[gidx // CT]
                        rhs = gtile[:, (gidx - c0) * D1:(gidx - c0 + 1) * D1]
                        nc.tensor.matmul(
                            out=ps[:], lhsT=sp3[:, toff + t], rhs=rhs,
                            start=(t == 0), stop=(t == WT - 1),
                        )
                    rec = clpool.tile([128, 1], F32)
                    nc.vector.reciprocal(rec[:], ps[:, d:D1])
                    nc.scalar.activation(
                        out=ob[:, j], in_=ps[:, 0:d],
                        func=(mybir.ActivationFunctionType.Relu if relu
                              else mybir.ActivationFunctionType.Copy),
                        scale=rec[:, :1],
                    )
                nc.sync.dma_start(out=ho[:, wb:wb + nb], in_=ob[:, 0:nb])
    nc.compile()
    return nc


def _build_link(pt):
    """Link predictor from host-staged row tiles:
    z = sigmoid(sum_f g0*wl0 + sum_f g1*wl1 + bl) for pt*128 pairs."""
    nc = bacc.Bacc(num_devices=NCORES)
    g0 = nc.dram_tensor("g0", [128, pt * F_IN], F16, kind="ExternalInput").ap()
    g1 = nc.dram_tensor("g1", [128, pt * F_IN], F16, kind="ExternalInput").ap()
    wl0 = nc.dram_tensor("wl0", [128, F_IN], F32, kind="ExternalInput").ap()
    wl1 = nc.dram_tensor("wl1", [128, F_IN], F32, kind="ExternalInput").ap()
    blr = nc.dram_tensor("blr", [128, 1], F32, kind="ExternalInput").ap()
    z = nc.dram_tensor("z", [128, pt], F32, kind="ExternalOutput").ap()

    with tile.TileContext(nc) as tc:
        with (
            tc.tile_pool(name="const", bufs=1) as cpool,
            tc.tile_pool(name="sc", bufs=6) as scpool,
        ):
            w0s = cpool.tile([128, F_IN], F32)
            nc.sync.dma_start(out=w0s[:], in_=wl0[:])
            w1s = cpool.tile([128, F_IN], F32)
            nc.sync.dma_start(out=w1s[:], in_=wl1[:])
            bls = cpool.tile([128, 1], F32)
            nc.sync.dma_start(out=bls[:], in_=blr[:])
            zsb = cpool.tile([128, pt], F32)

            g0s = cpool.tile([128, pt * F_IN], F16, tag="g0s")
            nc.sync.dma_start(out=g0s[:], in_=g0[:])
            g1s = cpool.tile([128, pt * F_IN], F16, tag="g1s")
            nc.sync.dma_start(out=g1s[:], in_=g1[:])

            for t in range(pt):
                scr = scpool.tile([128, F_IN], F32, tag="scr0")
                s0 = scpool.tile([128, 1], F32, tag="s0")
                nc.vector.scalar_tensor_tensor(
                    out=scr[:], in0=g0s[:, t * F_IN:(t + 1) * F_IN],
                    scalar=1.0, in1=w0s[:],
                    op0=mybir.AluOpType.mult, op1=mybir.AluOpType.mult,
                    accum_out=s0[:],
                )
                scr1 = scpool.tile([128, F_IN], F32, tag="scr1")
                s1 = scpool.tile([128, 1], F32, tag="s1")
                nc.vector.scalar_tensor_tensor(
                    out=scr1[:], in0=g1s[:, t * F_IN:(t + 1) * F_IN],
                    scalar=1.0, in1=w1s[:],
                    op0=mybir.AluOpType.mult, op1=mybir.AluOpType.mult,
                    accum_out=s1[:],
                )
                ssum = scpool.tile([128, 1], F32, tag="ss")
                nc.vector.tensor_tensor(
                    out=ssum[:], in0=s0[:], in1=s1[:], op=mybir.AluOpType.add
                )
                nc.scalar.activation(
                    out=zsb[:, t:t + 1], in_=ssum[:],
                    func=mybir.ActivationFunctionType.Sigmoid, bias=bls[:, :1],
                )
            nc.sync.dma_start(out=z[:], in_=zsb[:])
    nc.compile()
    return nc


def _run(name, nc, in_maps, trace=True):
    last = None
    for attempt in range(3):
        try:
            res = run_bass_kernel_spmd(
                nc, in_maps, core_ids=list(range(NCORES)), trace=trace
            )
            LAST_EXEC_NS[name] = res.exec_time_ns
            return res.results
        except Exception as e:  # wedged-device retry (clears on re-attempt)
            last = e
            time.sleep(5)
    raise last


def _rep(v, n=128):
    return np.ascontiguousarray(np.broadcast_to(np.asarray(v, np.float32), (n, len(v))))


def _tile_xT(xfull_shards, kc, d_in):
    """list of [NSP, d_in] fp16 per core -> [NCORES, kc, 128, W, 128] fp16
    (partition-major: xT[c,k,p,w,f] = x[w*128+f? no: x^T tiles)."""
    out = np.zeros((NCORES, kc, 128, W, 128), np.float16)
    for c in range(NCORES):
        xt = xfull_shards[c].T  # [d_in, NSP]
        for k in range(kc):
            blk = xt[128 * k:128 * (k + 1)].reshape(128, W, 128)
            out[c, k] = blk
    return out


# ------------------------------------------------------------------- kernel
def kernel(features, edge_index, mask, W1, a_src1, a_dst1, b1, W2, a_src2,
           a_dst2, b2, Wl, bl):
    features = np.asarray(features, np.float32)
    edge_index = np.asarray(edge_index, np.int32)
    mask = np.asarray(mask, np.int32)
    W1, W2, Wl = (np.asarray(a, np.float32) for a in (W1, W2, Wl))
    a_src1, a_dst1, b1 = (np.asarray(a, np.float32) for a in (a_src1, a_dst1, b1))
    a_src2, a_dst2, b2 = (np.asarray(a, np.float32) for a in (a_src2, a_dst2, b2))
    bl = np.asarray(bl, np.float32)

    g = _prep_graph(edge_index)
    T, WT = g["T"], g["WT"]

    key = (T, WT)
    if key not in _PROG_CACHE:
        _PROG_CACHE[key] = dict(
            p1=_build_proj(1, H),
            a1=_build_agg(H, T, WT, relu=True),
            p2=_build_proj(2, F_IN),
            a2=_build_agg(F_IN, T, WT, relu=False),
            lk=_build_link((10000 // NCORES + 127) // 128),
        )
    progs = _PROG_CACHE[key]

    # ---- L1: H1 = X @ W1 (sharded, window-permuted rows), es1/ed1
    xsh = []
    for c in range(NCORES):
        xs = np.zeros((NSP, F_IN), np.float16)
        r2n = g["row2node"][c]
        m = r2n >= 0
        xs[m] = features[r2n[m]]
        xsh.append(xs)
    xT1 = _tile_xT(xsh, 1, F_IN)
    W1h = W1.astype(np.float16)
    bex1 = np.concatenate([b1, [0.0, 0.0]]).astype(np.float16)[None, :]
    r1 = _run("p1", progs["p1"], [
        dict(xT=xT1[c], Wm=W1h, asr=_rep(a_src1), adr=_rep(a_dst1), bex=bex1)
        for c in range(NCORES)
    ])
    H1e = _full_from_shards([r1[c]["h16"] for c in range(NCORES)], g, H)
    esed1 = _full_from_shards(
        [r1[c]["esed"].reshape(128, W, 2) for c in range(NCORES)], g, 2)
    es1, ed1 = esed1[:, 0], esed1[:, 1]

    # ---- L2: aggregate layer 1 -> h1r = relu(agg) (b1 already in table rows)
    ins2 = []
    for c in range(NCORES):
        esx, edx = _expand(es1, ed1, g, c)
        ins2.append(dict(gt=_gt_tiles(H1e, g, c), eqm=g["eq8"][c],
                         esx=esx, edx=edx))
    r2 = _run("a1", progs["a1"], ins2)
    h1r = [np.ascontiguousarray(r2[c]["ho"].transpose(1, 0, 2)).reshape(NSP, H)
           for c in range(NCORES)]

    # ---- L3: H2 = h1r @ W2, es2/ed2
    xT2 = _tile_xT(h1r, 2, H)
    W2h = W2.astype(np.float16)
    bex2 = np.concatenate([b2, [0.0, 0.0]]).astype(np.float16)[None, :]
    r3 = _run("p2", progs["p2"], [
        dict(xT=xT2[c], Wm=W2h, asr=_rep(a_src2), adr=_rep(a_dst2), bex=bex2)
        for c in range(NCORES)
    ])
    H2e = _full_from_shards([r3[c]["h16"] for c in range(NCORES)], g, F_IN)
    esed2 = _full_from_shards(
        [r3[c]["esed"].reshape(128, W, 2) for c in range(NCORES)], g, 2)
    es2, ed2 = esed2[:, 0], esed2[:, 1]

    # ---- L4: aggregate layer 2 -> h2 = agg (b2 already in table rows)
    ins4 = []
    for c in range(NCORES):
        esx, edx = _expand(es2, ed2, g, c)
        ins4.append(dict(gt=_gt_tiles(H2e, g, c), eqm=g["eq8"][c],
                         esx=esx, edx=edx))
    r4 = _run("a2", progs["a2"], ins4)
    H2f = _full_from_shards([r4[c]["ho"] for c in range(NCORES)], g, F_IN)

    # ---- L5: link predictor (host-staged row tiles)
    P = mask.shape[0]
    pc = P // NCORES
    pt = (pc + 127) // 128
    mT = mask.T
    wl0 = _rep(Wl[:F_IN, 0])
    wl1 = _rep(Wl[F_IN:, 0])
    blr = np.full((128, 1), float(bl[0]), np.float32)
    ins5 = []
    for c in range(NCORES):
        m0 = np.zeros((128, pt), np.int64)
        m1 = np.zeros((128, pt), np.int64)
        s = np.arange(pc)
        m0[s % 128, s // 128] = mT[0][c * pc:(c + 1) * pc]
        m1[s % 128, s // 128] = mT[1][c * pc:(c + 1) * pc]
        g0 = H2f[m0].reshape(128, pt * F_IN)
        g1 = H2f[m1].reshape(128, pt * F_IN)
        ins5.append(dict(g0=g0, g1=g1, wl0=wl0, wl1=wl1, blr=blr))
    r5 = _run("lk", progs["lk"], ins5)
    out = np.zeros((P, 1), np.float32)
    for c in range(NCORES):
        s = np.arange(pc)
        out[c * pc:(c + 1) * pc, 0] = r5[c]["z"][s % 128, s // 128]

    tot = sum(v for v in LAST_EXEC_NS.values() if v)
    print(f"kernel launches ns: {LAST_EXEC_NS} total {tot}")
    return out


# revision 22
# speedup vs baseline: 1.0968x; 1.0968x over previous
"""Two-layer GAT (single-head, PyG-style) + link predictor on 8 TRN2 NeuronCores.

Strategy (memory-regime):
  - Nodes sharded 8-way by id (6250/core, padded to 6272 = 49 windows of 128).
    Within a core, nodes are packed into windows by (in-degree+1) greedy
    bin-packing so every window holds <= 128 nodes and <= 128*WT edge slots;
    all windows share a uniform tile count WT (slot-major [128, T] layout).
  - Self-loops are ordinary edge slots (src == dst). Edge softmax needs no
    max-shift (the shift cancels in the ratio; logits are O(10)).
  - Halo exchange runs between launches on the host as pure index-space
    movement: per-edge source-feature tiles gt[p,t,:] = [H[src[p,t]] | 1.0]
    and per-edge es/ed scalars are assembled with numpy fancy indexing and
    staged as kernel inputs; the device streams them with large sequential
    DMAs (no indirect gathers). All floating-point math (projections,
    exp/leaky-relu, softmax-weighted scatter via one-hot matmuls,
    normalization, link predictor) happens on device.
  - The trailing 1.0 column of every edge row makes the same one-hot matmul
    accumulate the softmax denominator:
        ps[dst, 0:d] += sum_e p_e [dstrow_e == dst] H[src_e]
        ps[dst, d]   += sum_e p_e [dstrow_e == dst]
    One-hot tiles are built per-window in one batched DVE/Pool op pair using
    stride-0 broadcast access patterns; a per-window epilogue normalizes,
    adds bias (and relu for layer 1).
  - Dense projections run sharded on PE in fp16; es = h@a_s / ed = h@a_d come
    free as two extra matmul columns [W | W@a_s | W@a_d].

Launches: L1 proj1 -> L2 agg1 -> L3 proj2 -> L4 agg2 -> L5 link predictor.
"""
import heapq
import time

import numpy as np

import concourse.bass as bass
import concourse.mybir as mybir
import concourse.tile as tile
from concourse import bacc
from concourse.bass_utils import run_bass_kernel_spmd

F32 = mybir.dt.float32
F16 = mybir.dt.float16
F8 = mybir.dt.float8e4
I32 = mybir.dt.int32

NCORES = 8
N, F_IN, H, C = 50000, 128, 256, 1
NS = N // NCORES            # 6250 nodes per shard
W = (NS + 127) // 128       # 49 windows per shard
NSP = W * 128               # 6272 padded slots
NEG = -1.0e30               # pad-edge sentinel (exp -> exactly 0)
CHW = 3                     # windows per streaming DMA chunk
SPG = 3                     # windows per batched one-hot mult
WB = 7                      # windows per batched output write (49 = 7*7)

LAST_EXEC_NS = {}           # launch name -> exec_time_ns (filled per kernel() call)
_PROG_CACHE = {}


# ----------------------------------------------------------------- host prep
def _prep_graph(edge_index):
    """Per core: pack nodes into 49 windows by (deg+1) so all windows fit in
    <=128 nodes and a uniform WT tiles of 128 edge slots; lay self-loop +
    incoming edges of each window into slot-major [128, T] layout."""
    src = np.asarray(edge_index[0], np.int64)
    dst = np.asarray(edge_index[1], np.int64)
    deg = np.bincount(dst, minlength=N)

    order = np.argsort(dst, kind="stable")
    src_s = src[order]
    estart = np.concatenate([[0], np.cumsum(deg)])

    win_nodes = np.full((NCORES, W, 128), -1, np.int64)
    win_count = np.zeros((NCORES, W), np.int64)
    win_load = np.zeros((NCORES, W), np.int64)
    for c in range(NCORES):
        nodes = np.arange(c * NS, (c + 1) * NS)
        wgt = deg[nodes] + 1
        ordn = np.argsort(-wgt, kind="stable")
        heap = [(0, w) for w in range(W)]
        heapq.heapify(heap)
        skipped = []
        for i in ordn:
            n, gw = nodes[i], wgt[i]
            while True:
                load, w = heapq.heappop(heap)
                if win_count[c, w] < 128:
                    break
                skipped.append((load, w))
            for it in skipped:
                heapq.heappush(heap, it)
            skipped = []
            win_nodes[c, w, win_count[c, w]] = n
            win_count[c, w] += 1
            win_load[c, w] = load + gw
            heapq.heappush(heap, (load + gw, w))
    WT = max(int(np.ceil(win_load.max() / 128)), 1)
    T = W * WT

    srcs = np.zeros((NCORES, 128, T), np.int32)
    srcg = np.zeros((NCORES, 128, T), np.int64)
    dstg = np.zeros((NCORES, 128, T), np.int64)
    dstr = np.full((NCORES, 128, T), 128, np.int64)   # local dst row, 128 = pad
    pad = np.ones((NCORES, 128, T), bool)
    row2node = np.full((NCORES, NSP), -1, np.int64)

    for c in range(NCORES):
        for w in range(W):
            cnt = int(win_count[c, w])
            nl = win_nodes[c, w, :cnt]
            row2node[c, w * 128:w * 128 + cnt] = nl
            seg_src, seg_row = [], []
            for r, n in enumerate(nl):
                e0, e1 = int(estart[n]), int(estart[n + 1])
                ss = np.concatenate([[n], src_s[e0:e1]])
                seg_src.append(ss)
                seg_row.append(np.full(len(ss), r, np.int64))
            ss = np.concatenate(seg_src)
            rr = np.concatenate(seg_row)
            sl = np.arange(len(ss))
            pp, tt = sl % 128, w * WT + sl // 128
            srcs[c, pp, tt] = ss
            srcg[c, pp, tt] = ss
            dstg[c, pp, tt] = nl[rr]
            dstr[c, pp, tt] = rr
            pad[c, pp, tt] = False
    # fp8 one-hot mask (row 128 of eyeZ = zeros for pads); shared by both layers
    f8np = mybir.dt.np(F8)
    eyeZ = np.zeros((129, 128), f8np)
    eyeZ[np.arange(128), np.arange(128)] = 1.0
    eq8 = eyeZ[dstr]                                   # [NCORES, 128, T, 128]
    return dict(srcs=srcs, srcg=srcg, dstg=dstg, eq8=eq8, pad=pad,
                row2node=row2node, WT=WT, T=T)


def _expand(es_full, ed_full, g, c):
    esx = es_full[g["srcg"][c]].astype(np.float32)
    edx = ed_full[g["dstg"][c]].astype(np.float32)
    p = g["pad"][c]
    esx[p] = NEG
    edx[p] = 0.0
    return esx, edx


def _full_from_shards(shards, g, cols):
    """shards: per-core [128, W, cols] (row w*128+p) -> node-indexed [N, cols]."""
    out = np.zeros((N, cols), shards[0].dtype)
    for c in range(NCORES):
        flat = np.ascontiguousarray(shards[c].transpose(1, 0, 2)).reshape(NSP, cols)
        r2n = g["row2node"][c]
        m = r2n >= 0
        out[r2n[m]] = flat[m]
    return out


def _gt_tiles(Hfull, g, c):
    """[128, T*(d+1)] fp16 edge tiles [H[src] | 1.0]."""
    d = Hfull.shape[1]
    gt = np.ones((128, g["T"], d + 1), np.float16)
    gt[:, :, :d] = Hfull[g["srcs"][c]]
    return gt.reshape(128, g["T"] * (d + 1))


# ------------------------------------------------------------- bass programs
def _build_proj(kc, d_out):
    """Projection: psum = bias_ext + x @ [W | W@a_s | W@a_d] per 128-node
    window. The layer bias rides in via a rank-1 ones-row matmul (softmax
    weights sum to 1, so adding b to every table row equals adding b after
    aggregation); its es/ed columns are zero so the attention dots stay
    bias-free. xT fp16 [kc, 128, W, 128], Wm fp16 [kc*128, d_out],
    asr/adr fp32 [128, d_out], bex fp16 [1, d_out+2] = [b | 0 0].
    Outputs h16 [128, W, d_out] fp16, esed [128, 2W] f32 (interleaved es,ed)."""
    nc = bacc.Bacc(num_devices=NCORES)
    xT = nc.dram_tensor("xT", [kc, 128, W, 128], F16, kind="ExternalInput").ap()
    Wm = nc.dram_tensor("Wm", [kc * 128, d_out], F16, kind="ExternalInput").ap()
    asr = nc.dram_tensor("asr", [128, d_out], F32, kind="ExternalInput").ap()
    adr = nc.dram_tensor("adr", [128, d_out], F32, kind="ExternalInput").ap()
    bex = nc.dram_tensor("bex", [1, d_out + 2], F16, kind="ExternalInput").ap()
    h16 = nc.dram_tensor("h16", [128, W, d_out], F16, kind="ExternalOutput").ap()
    esed = nc.dram_tensor("esed", [128, 2 * W], F32, kind="ExternalOutput").ap()

    with tile.TileContext(nc) as tc:
        with (
            tc.tile_pool(name="const", bufs=1) as cpool,
            tc.tile_pool(name="o", bufs=3) as opool,
            tc.tile_pool(name="ps", bufs=4, space="PSUM") as pspool,
            tc.tile_pool(name="sc", bufs=4) as scpool,
        ):
            asb = cpool.tile([128, d_out], F32)
            nc.sync.dma_start(out=asb[:], in_=asr[:])
            adb = cpool.tile([128, d_out], F32)
            nc.sync.dma_start(out=adb[:], in_=adr[:])
            bxb = cpool.tile([1, d_out + 2], F16, tag="bx")
            nc.sync.dma_start(out=bxb[:], in_=bex[:])
            one1 = cpool.tile([1, 128], F16, tag="one1")
            nc.vector.memset(one1[:], 1.0)
            esedb = cpool.tile([128, 2 * W], F32)

            xls = []
            for k in range(kc):
                xl = cpool.tile([128, W, 128], F16, tag=f"x{k}")
                nc.sync.dma_start(out=xl[:], in_=xT[k])
                xls.append(xl)

            wsb = []
            for k in range(kc):
                wk = cpool.tile([128, d_out + 2], F16, tag=f"w{k}")
                nc.sync.dma_start(
                    out=wk[:, 0:d_out], in_=Wm[128 * k:128 * (k + 1), :]
                )
                scr = scpool.tile([128, d_out], F32, tag="wes")
                nc.vector.tensor_tensor(
                    out=scr[:], in0=wk[:, 0:d_out], in1=asb[:],
                    op=mybir.AluOpType.mult,
                )
                wes = scpool.tile([128, 1], F32, tag="wesc")
                nc.vector.reduce_sum(
                    out=wes[:], in_=scr[:], axis=mybir.AxisListType.X
                )
                nc.vector.tensor_copy(out=wk[:, d_out:d_out + 1], in_=wes[:])
                scr2 = scpool.tile([128, d_out], F32, tag="wed")
                nc.vector.tensor_tensor(
                    out=scr2[:], in0=wk[:, 0:d_out], in1=adb[:],
                    op=mybir.AluOpType.mult,
                )
                wed = scpool.tile([128, 1], F32, tag="wedc")
                nc.vector.reduce_sum(
                    out=wed[:], in_=scr2[:], axis=mybir.AxisListType.X
                )
                nc.vector.tensor_copy(out=wk[:, d_out + 1:d_out + 2], in_=wed[:])
                wsb.append(wk)

            for wb in range(0, W, WB):
                nb = min(WB, W - wb)
                ob = opool.tile([128, WB, d_out], F16)
                for j in range(nb):
                    w = wb + j
                    ps = pspool.tile([128, d_out + 2], F32, space="PSUM")
                    nc.tensor.matmul(
                        out=ps[:], lhsT=one1[:], rhs=bxb[:],
                        start=True, stop=False,
                    )
                    for k in range(kc):
                        nc.tensor.matmul(
                            out=ps[:], lhsT=xls[k][:, w], rhs=wsb[k][:],
                            start=False, stop=(k == kc - 1),
                        )
                    nc.scalar.activation(
                        out=ob[:, j], in_=ps[:, 0:d_out],
                        func=mybir.ActivationFunctionType.Copy,
                    )
                    nc.vector.tensor_copy(
                        out=esedb[:, 2 * w:2 * w + 2],
                        in_=ps[:, d_out:d_out + 2],
                    )
                nc.sync.dma_start(
                    out=h16[:, wb:wb + nb], in_=ob[:, 0:nb]
                )
            nc.sync.dma_start(out=esed[:], in_=esedb[:])
    nc.compile()
    return nc


def _build_agg(d, T, WT, relu):
    """Aggregation over one GAT layer from host-staged edge tiles.
    gt [128, T*(d+1)] f16 ([H[src]+b | 1] edge rows), eq8 [128, T, 128] fp8
    one-hot dst masks, esx/edx [128, T] f32 -> ho [128, W, d] f16.
    Per window: sp = eq8 * p (one DVE op), WT one-hot matmuls accumulate
    [sum p*h | sum p] in PSUM, epilogue scales by 1/sum p on ACT."""
    D1 = d + 1
    nc = bacc.Bacc(num_devices=NCORES)
    gt = nc.dram_tensor("gt", [128, T * D1], F16, kind="ExternalInput").ap()
    eqm = nc.dram_tensor("eqm", [128, T, 128], F8, kind="ExternalInput").ap()
    esx = nc.dram_tensor("esx", [128, T], F32, kind="ExternalInput").ap()
    edx = nc.dram_tensor("edx", [128, T], F32, kind="ExternalInput").ap()
    ho = nc.dram_tensor("ho", [128, W, d], F16, kind="ExternalOutput").ap()

    CT = CHW * WT                       # tiles per stream chunk
    nchunk = (W + CHW - 1) // CHW
    with tile.TileContext(nc) as tc:
        with (
            tc.tile_pool(name="const", bufs=1) as cpool,
            tc.tile_pool(name="g", bufs=3) as gpool,
            tc.tile_pool(name="e", bufs=3) as epool,
            tc.tile_pool(name="sp", bufs=4) as sppool,
            tc.tile_pool(name="o", bufs=3) as opool,
            tc.tile_pool(name="cl", bufs=4) as clpool,
            tc.tile_pool(name="ps", bufs=4, space="PSUM") as pspool,
        ):
            esxs = cpool.tile([128, T], F32)
            nc.sync.dma_start(out=esxs[:], in_=esx[:])
            edxs = cpool.tile([128, T], F32)
            nc.sync.dma_start(out=edxs[:], in_=edx[:])

            # p = exp(leakyrelu(es+ed, 0.2)) in fp16
            lg = cpool.tile([128, T], F32, tag="lg")
            nc.vector.tensor_tensor(
                out=lg[:], in0=esxs[:], in1=edxs[:], op=mybir.AluOpType.add
            )
            lg2 = cpool.tile([128, T], F32, tag="lg2")
            nc.vector.tensor_scalar_mul(out=lg2[:], in0=lg[:], scalar1=0.2)
            nc.vector.tensor_tensor(
                out=lg[:], in0=lg[:], in1=lg2[:], op=mybir.AluOpType.max
            )
            p16 = cpool.tile([128, T], F16, tag="p16")
            nc.scalar.activation(
                out=p16[:], in_=lg[:], func=mybir.ActivationFunctionType.Exp
            )

            gts, eqs = [], []
            for ci in range(nchunk):
                c0, c1 = ci * CT, min((ci + 1) * CT, T)
                gtile = gpool.tile([128, (c1 - c0) * D1], F16)
                nc.sync.dma_start(out=gtile[:], in_=gt[:, c0 * D1:c1 * D1])
                gts.append((gtile, c0))
                etile = epool.tile([128, c1 - c0, 128], F8)
                nc.sync.dma_start(out=etile[:], in_=eqm[:, c0:c1])
                eqs.append((etile, c0))

            # one-hot mults run over SPG-window groups (chunk-aligned since
            # CHW % SPG == 0 or SPG % CHW == 0 keeps groups within... groups
            # must not span stream chunks: CHW == SPG ensures alignment.
            sps = {}
            for wb in range(0, W, WB):
                nb = min(WB, W - wb)
                ob = opool.tile([128, WB, d], F16)
                for j in range(nb):
                    w = wb + j
                    t0 = w * WT
                    if w % SPG == 0:
                        ng = min(SPG, W - w)
                        # sp[p, t, q] = eq8[p, t0+t, q] * p16[p, t0+t]
                        sp3 = sppool.tile([128, SPG * WT, 128], F16)
                        etile, e0 = eqs[(t0 // CT)]
                        p_b = p16[:, t0:t0 + ng * WT].unsqueeze(2).to_broadcast(
                            [128, ng * WT, 128])
                        nc.vector.tensor_tensor(
                            out=sp3[:, 0:ng * WT],
                            in0=etile[:, t0 - e0:t0 - e0 + ng * WT],
                            in1=p_b, op=mybir.AluOpType.mult,
                        )
                        sps[w] = sp3
                    sp3 = sps[w - w % SPG]
                    toff = (w % SPG) * WT
                    ps = pspool.tile([128, D1], F32, space="PSUM")
                    for t in range(WT):
                        gidx = t0 + t
                        gtile, c0 = gts[gidx // CT]
                        rhs = gtile[:, (gidx - c0) * D1:(gidx - c0 + 1) * D1]
                        nc.tensor.matmul(
                            out=ps[:], lhsT=sp3[:, toff + t], rhs=rhs,
                            start=(t == 0), stop=(t == WT - 1),
                        )
                    rec = clpool.tile([128, 1], F32)
                    nc.vector.reciprocal(rec[:], ps[:, d:D1])
                    nc.scalar.activation(
                        out=ob[:, j], in_=ps[:, 0:d],
                        func=(mybir.ActivationFunctionType.Relu if relu
                              else mybir.ActivationFunctionType.Copy),
                        scale=rec[:, :1],
                    )
                nc.sync.dma_start(out=ho[:, wb:wb + nb], in_=ob[:, 0:nb])
    nc.compile()
    return nc


def _build_link(pt):
    """Link predictor from host-staged row tiles:
    z = sigmoid(sum_f g0*wl0 + sum_f g1*wl1 + bl) for pt*128 pairs."""
    nc = bacc.Bacc(num_devices=NCORES)
    g0 = nc.dram_tensor("g0", [128, pt * F_IN], F16, kind="ExternalInput").ap()
    g1 = nc.dram_tensor("g1", [128, pt * F_IN], F16, kind="ExternalInput").ap()
    wl0 = nc.dram_tensor("wl0", [128, F_IN], F32, kind="ExternalInput").ap()
    wl1 = nc.dram_tensor("wl1", [128, F_IN], F32, kind="ExternalInput").ap()
    blr = nc.dram_tensor("blr", [128, 1], F32, kind="ExternalInput").ap()
    z = nc.dram_tensor("z", [128, pt], F32, kind="ExternalOutput").ap()

    with tile.TileContext(nc) as tc:
        with (
            tc.tile_pool(name="const", bufs=1) as cpool,
            tc.tile_pool(name="sc", bufs=6) as scpool,
        ):
            w0s = cpool.tile([128, F_IN], F32)
            nc.sync.dma_start(out=w0s[:], in_=wl0[:])
            w1s = cpool.tile([128, F_IN], F32)
            nc.sync.dma_start(out=w1s[:], in_=wl1[:])
            bls = cpool.tile([128, 1], F32)
            nc.sync.dma_start(out=bls[:], in_=blr[:])
            zsb = cpool.tile([128, pt], F32)

            g0s = cpool.tile([128, pt * F_IN], F16, tag="g0s")
            nc.sync.dma_start(out=g0s[:], in_=g0[:])
            g1s = cpool.tile([128, pt * F_IN], F16, tag="g1s")
            nc.sync.dma_start(out=g1s[:], in_=g1[:])

            for t in range(pt):
                scr = scpool.tile([128, F_IN], F32, tag="scr0")
                s0 = scpool.tile([128, 1], F32, tag="s0")
                nc.vector.scalar_tensor_tensor(
                    out=scr[:], in0=g0s[:, t * F_IN:(t + 1) * F_IN],
                    scalar=1.0, in1=w0s[:],
                    op0=mybir.AluOpType.mult, op1=mybir.AluOpType.mult,
                    accum_out=s0[:],
                )
                scr1 = scpool.tile([128, F_IN], F32, tag="scr1")
                s1 = scpool.tile([128, 1], F32, tag="s1")
                nc.vector.scalar_tensor_tensor(
                    out=scr1[:], in0=g1s[:, t * F_IN:(t + 1) * F_IN],
                    scalar=1.0, in1=w1s[:],
                    op0=mybir.AluOpType.mult, op1=mybir.AluOpType.mult,
                    accum_out=s1[:],
                )
                ssum = scpool.tile([128, 1], F32, tag="ss")
                nc.vector.tensor_tensor(
                    out=ssum[:], in0=s0[:], in1=s1[:], op=mybir.AluOpType.add
                )
                nc.scalar.activation(
                    out=zsb[:, t:t + 1], in_=ssum[:],
                    func=mybir.ActivationFunctionType.Sigmoid, bias=bls[:, :1],
                )
            nc.sync.dma_start(out=z[:], in_=zsb[:])
    nc.compile()
    return nc


def _run(name, nc, in_maps, trace=True):
    last = None
    for attempt in range(3):
        try:
            res = run_bass_kernel_spmd(
                nc, in_maps, core_ids=list(range(NCORES)), trace=trace
            )
            LAST_EXEC_NS[name] = res.exec_time_ns
            return res.results
        except Exception as e:  # wedged-device retry (clears on re-attempt)
            last = e
            time.sleep(5)
    raise last


def _rep(v, n=128):
    return np.ascontiguousarray(np.broadcast_to(np.asarray(v, np.float32), (n, len(v))))


def _tile_xT(xfull_shards, kc, d_in):
    """list of [NSP, d_in] fp16 per core -> [NCORES, kc, 128, W, 128] fp16
    (partition-major: xT[c,k,p,w,f] = x[w*128+f? no: x^T tiles)."""
    out = np.zeros((NCORES, kc, 128, W, 128), np.float16)
    for c in range(NCORES):
        xt = xfull_shards[c].T  # [d_in, NSP]
        for k in range(kc):
            blk = xt[128 * k:128 * (k + 1)].reshape(128, W, 128)
            out[c, k] = blk
    return out


# ------------------------------------------------------------------- kernel
def kernel(features, edge_index, mask, W1, a_src1, a_dst1, b1, W2, a_src2,
           a_dst2, b2, Wl, bl):
    features = np.asarray(features, np.float32)
    edge_index = np.asarray(edge_index, np.int32)
    mask = np.asarray(mask, np.int32)
    W1, W2, Wl = (np.asarray(a, np.float32) for a in (W1, W2, Wl))
    a_src1, a_dst1, b1 = (np.asarray(a, np.float32) for a in (a_src1, a_dst1, b1))
    a_src2, a_dst2, b2 = (np.asarray(a, np.float32) for a in (a_src2, a_dst2, b2))
    bl = np.asarray(bl, np.float32)

    g = _prep_graph(edge_index)
    T, WT = g["T"], g["WT"]

    key = (T, WT)
    if key not in _PROG_CACHE:
        _PROG_CACHE[key] = dict(
            p1=_build_proj(1, H),
            a1=_build_agg(H, T, WT, relu=True),
            p2=_build_proj(2, F_IN),
            a2=_build_agg(F_IN, T, WT, relu=False),
            lk=_build_link((10000 // NCORES + 127) // 128),
        )
    progs = _PROG_CACHE[key]

    # ---- L1: H1 = X @ W1 (sharded, window-permuted rows), es1/ed1
    xsh = []
    for c in range(NCORES):
        xs = np.zeros((NSP, F_IN), np.float16)
        r2n = g["row2node"][c]
        m = r2n >= 0
        xs[m] = features[r2n[m]]
        xsh.append(xs)
    xT1 = _tile_xT(xsh, 1, F_IN)
    W1h = W1.astype(np.float16)
    bex1 = np.concatenate([b1, [0.0, 0.0]]).astype(np.float16)[None, :]
    r1 = _run("p1", progs["p1"], [
        dict(xT=xT1[c], Wm=W1h, asr=_rep(a_src1), adr=_rep(a_dst1), bex=bex1)
        for c in range(NCORES)
    ])
    H1e = _full_from_shards([r1[c]["h16"] for c in range(NCORES)], g, H)
    esed1 = _full_from_shards(
        [r1[c]["esed"].reshape(128, W, 2) for c in range(NCORES)], g, 2)
    es1, ed1 = esed1[:, 0], esed1[:, 1]

    # ---- L2: aggregate layer 1 -> h1r = relu(agg) (b1 already in table rows)
    ins2 = []
    for c in range(NCORES):
        esx, edx = _expand(es1, ed1, g, c)
        ins2.append(dict(gt=_gt_tiles(H1e, g, c), eqm=g["eq8"][c],
                         esx=esx, edx=edx))
    r2 = _run("a1", progs["a1"], ins2)
    h1r = [np.ascontiguousarray(r2[c]["ho"].transpose(1, 0, 2)).reshape(NSP, H)
           for c in range(NCORES)]

    # ---- L3: H2 = h1r @ W2, es2/ed2
    xT2 = _tile_xT(h1r, 2, H)
    W2h = W2.astype(np.float16)
    bex2 = np.concatenate([b2, [0.0, 0.0]]).astype(np.float16)[None, :]
    r3 = _run("p2", progs["p2"], [
        dict(xT=xT2[c], Wm=W2h, asr=_rep(a_src2), adr=_rep(a_dst2), bex=bex2)
        for c in range(NCORES)
    ])
    H2e = _full_from_shards([r3[c]["h16"] for c in range(NCORES)], g, F_IN)
    esed2 = _full_from_shards(
        [r3[c]["esed"].reshape(128, W, 2) for c in range(NCORES)], g, 2)
    es2, ed2 = esed2[:, 0], esed2[:, 1]

    # ---- L4: aggregate layer 2 -> h2 = agg (b2 already in table rows)
    ins4 = []
    for c in range(NCORES):
        esx, edx = _expand(es2, ed2, g, c)
        ins4.append(dict(gt=_gt_tiles(H2e, g, c), eqm=g["eq8"][c],
                         esx=esx, edx=edx))
    r4 = _run("a2", progs["a2"], ins4)
    H2f = _full_from_shards([r4[c]["ho"] for c in range(NCORES)], g, F_IN)

    # ---- L5: link predictor (host-staged row tiles)
    P = mask.shape[0]
    pc = P // NCORES
    pt = (pc + 127) // 128
    mT = mask.T
    wl0 = _rep(Wl[:F_IN, 0])
    wl1 = _rep(Wl[F_IN:, 0])
    blr = np.full((128, 1), float(bl[0]), np.float32)
    ins5 = []
    for c in range(NCORES):
        m0 = np.zeros((128, pt), np.int64)
        m1 = np.zeros((128, pt), np.int64)
        s = np.arange(pc)
        m0[s % 128, s // 128] = mT[0][c * pc:(c + 1) * pc]
        m1[s % 128, s // 128] = mT[1][c * pc:(c + 1) * pc]
        g0 = H2f[m0].reshape(128, pt * F_IN)
        g1 = H2f[m1].reshape(128, pt * F_IN)
        ins5.append(dict(g0=g0, g1=g1, wl0=wl0, wl1=wl1, blr=blr))
    r5 = _run("lk", progs["lk"], ins5)
    out = np.zeros((P, 1), np.float32)
    for c in range(NCORES):
        s = np.arange(pc)
        out[c * pc:(c + 1) * pc, 0] = r5[c]["z"][s % 128, s // 128]

    tot = sum(v for v in LAST_EXEC_NS.values() if v)
    print(f"kernel launches ns: {LAST_EXEC_NS} total {tot}")
    return out


# revision 23
# speedup vs baseline: 1.1000x; 1.0029x over previous
"""Two-layer GAT (single-head, PyG-style) + link predictor on 8 TRN2 NeuronCores.

Strategy (memory-regime):
  - Nodes sharded 8-way by id (6250/core, padded to 6272 = 49 windows of 128).
    Within a core, nodes are packed into windows by (in-degree+1) greedy
    bin-packing so every window holds <= 128 nodes and <= 128*WT edge slots;
    all windows share a uniform tile count WT (slot-major [128, T] layout).
  - Self-loops are ordinary edge slots (src == dst). Edge softmax needs no
    max-shift (the shift cancels in the ratio; logits are O(10)).
  - Halo exchange runs between launches on the host as pure index-space
    movement: per-edge source-feature tiles gt[p,t,:] = [H[src[p,t]] | 1.0]
    and per-edge es/ed scalars are assembled with numpy fancy indexing and
    staged as kernel inputs; the device streams them with large sequential
    DMAs (no indirect gathers). All floating-point math (projections,
    exp/leaky-relu, softmax-weighted scatter via one-hot matmuls,
    normalization, link predictor) happens on device.
  - The trailing 1.0 column of every edge row makes the same one-hot matmul
    accumulate the softmax denominator:
        ps[dst, 0:d] += sum_e p_e [dstrow_e == dst] H[src_e]
        ps[dst, d]   += sum_e p_e [dstrow_e == dst]
    One-hot tiles are built per-window in one batched DVE/Pool op pair using
    stride-0 broadcast access patterns; a per-window epilogue normalizes,
    adds bias (and relu for layer 1).
  - Dense projections run sharded on PE in fp16; es = h@a_s / ed = h@a_d come
    free as two extra matmul columns [W | W@a_s | W@a_d].

Launches: L1 proj1 -> L2 agg1 -> L3 proj2 -> L4 agg2 -> L5 link predictor.
"""
import heapq
import time

import numpy as np

import concourse.bass as bass
import concourse.mybir as mybir
import concourse.tile as tile
from concourse import bacc
from concourse.bass_utils import run_bass_kernel_spmd

F32 = mybir.dt.float32
F16 = mybir.dt.float16
F8 = mybir.dt.float8e4
I32 = mybir.dt.int32

NCORES = 8
N, F_IN, H, C = 50000, 128, 256, 1
NS = N // NCORES            # 6250 nodes per shard
W = (NS + 127) // 128       # 49 windows per shard
NSP = W * 128               # 6272 padded slots
NEG = -1.0e30               # pad-edge sentinel (exp -> exactly 0)
CHW = 3                     # windows per streaming DMA chunk
SPG = 3                     # windows per batched one-hot mult
WB = 7                      # windows per batched output write (49 = 7*7)

LAST_EXEC_NS = {}           # launch name -> exec_time_ns (filled per kernel() call)
_PROG_CACHE = {}


# ----------------------------------------------------------------- host prep
def _prep_graph(edge_index):
    """Per core: pack nodes into 49 windows by (deg+1) so all windows fit in
    <=128 nodes and a uniform WT tiles of 128 edge slots; lay self-loop +
    incoming edges of each window into slot-major [128, T] layout."""
    src = np.asarray(edge_index[0], np.int64)
    dst = np.asarray(edge_index[1], np.int64)
    deg = np.bincount(dst, minlength=N)

    order = np.argsort(dst, kind="stable")
    src_s = src[order]
    estart = np.concatenate([[0], np.cumsum(deg)])

    win_nodes = np.full((NCORES, W, 128), -1, np.int64)
    win_count = np.zeros((NCORES, W), np.int64)
    win_load = np.zeros((NCORES, W), np.int64)
    for c in range(NCORES):
        nodes = np.arange(c * NS, (c + 1) * NS)
        wgt = deg[nodes] + 1
        ordn = np.argsort(-wgt, kind="stable")
        heap = [(0, w) for w in range(W)]
        heapq.heapify(heap)
        skipped = []
        for i in ordn:
            n, gw = nodes[i], wgt[i]
            while True:
                load, w = heapq.heappop(heap)
                if win_count[c, w] < 128:
                    break
                skipped.append((load, w))
            for it in skipped:
                heapq.heappush(heap, it)
            skipped = []
            win_nodes[c, w, win_count[c, w]] = n
            win_count[c, w] += 1
            win_load[c, w] = load + gw
            heapq.heappush(heap, (load + gw, w))
    WT = max(int(np.ceil(win_load.max() / 128)), 1)
    T = W * WT

    srcs = np.zeros((NCORES, 128, T), np.int32)
    srcg = np.zeros((NCORES, 128, T), np.int64)
    dstg = np.zeros((NCORES, 128, T), np.int64)
    dstr = np.full((NCORES, 128, T), 128, np.int64)   # local dst row, 128 = pad
    pad = np.ones((NCORES, 128, T), bool)
    row2node = np.full((NCORES, NSP), -1, np.int64)

    for c in range(NCORES):
        for w in range(W):
            cnt = int(win_count[c, w])
            nl = win_nodes[c, w, :cnt]
            row2node[c, w * 128:w * 128 + cnt] = nl
            seg_src, seg_row = [], []
            for r, n in enumerate(nl):
                e0, e1 = int(estart[n]), int(estart[n + 1])
                ss = np.concatenate([[n], src_s[e0:e1]])
                seg_src.append(ss)
                seg_row.append(np.full(len(ss), r, np.int64))
            ss = np.concatenate(seg_src)
            rr = np.concatenate(seg_row)
            sl = np.arange(len(ss))
            pp, tt = sl % 128, w * WT + sl // 128
            srcs[c, pp, tt] = ss
            srcg[c, pp, tt] = ss
            dstg[c, pp, tt] = nl[rr]
            dstr[c, pp, tt] = rr
            pad[c, pp, tt] = False
    # fp8 one-hot mask (row 128 of eyeZ = zeros for pads); shared by both layers
    f8np = mybir.dt.np(F8)
    eyeZ = np.zeros((129, 128), f8np)
    eyeZ[np.arange(128), np.arange(128)] = 1.0
    eq8 = eyeZ[dstr]                                   # [NCORES, 128, T, 128]
    return dict(srcs=srcs, srcg=srcg, dstg=dstg, eq8=eq8, pad=pad,
                row2node=row2node, WT=WT, T=T)


def _expand(es_full, ed_full, g, c):
    esx = es_full[g["srcg"][c]].astype(np.float32)
    edx = ed_full[g["dstg"][c]].astype(np.float32)
    p = g["pad"][c]
    esx[p] = NEG
    edx[p] = 0.0
    return esx, edx


def _full_from_shards(shards, g, cols):
    """shards: per-core [128, W, cols] (row w*128+p) -> node-indexed [N, cols]."""
    out = np.zeros((N, cols), shards[0].dtype)
    for c in range(NCORES):
        flat = np.ascontiguousarray(shards[c].transpose(1, 0, 2)).reshape(NSP, cols)
        r2n = g["row2node"][c]
        m = r2n >= 0
        out[r2n[m]] = flat[m]
    return out


def _gt_tiles(Hfull, g, c):
    """[128, T*(d+1)] fp8 edge tiles [H[src] | 1.0] (1.0 exact in e4m3)."""
    d = Hfull.shape[1]
    gt = np.ones((128, g["T"], d + 1), mybir.dt.np(F8))
    gt[:, :, :d] = Hfull[g["srcs"][c]].astype(mybir.dt.np(F8))
    return gt.reshape(128, g["T"] * (d + 1))


# ------------------------------------------------------------- bass programs
def _build_proj(kc, d_out):
    """Projection: psum = bias_ext + x @ [W | W@a_s | W@a_d] per 128-node
    window. The layer bias rides in via a rank-1 ones-row matmul (softmax
    weights sum to 1, so adding b to every table row equals adding b after
    aggregation); its es/ed columns are zero so the attention dots stay
    bias-free. xT fp16 [kc, 128, W, 128], Wm fp16 [kc*128, d_out],
    asr/adr fp32 [128, d_out], bex fp16 [1, d_out+2] = [b | 0 0].
    Outputs h16 [128, W, d_out] fp16, esed [128, 2W] f32 (interleaved es,ed)."""
    nc = bacc.Bacc(num_devices=NCORES)
    xT = nc.dram_tensor("xT", [kc, 128, W, 128], F16, kind="ExternalInput").ap()
    Wm = nc.dram_tensor("Wm", [kc * 128, d_out], F16, kind="ExternalInput").ap()
    asr = nc.dram_tensor("asr", [128, d_out], F32, kind="ExternalInput").ap()
    adr = nc.dram_tensor("adr", [128, d_out], F32, kind="ExternalInput").ap()
    bex = nc.dram_tensor("bex", [1, d_out + 2], F16, kind="ExternalInput").ap()
    h16 = nc.dram_tensor("h16", [128, W, d_out], F16, kind="ExternalOutput").ap()
    esed = nc.dram_tensor("esed", [128, 2 * W], F32, kind="ExternalOutput").ap()

    with tile.TileContext(nc) as tc:
        with (
            tc.tile_pool(name="const", bufs=1) as cpool,
            tc.tile_pool(name="o", bufs=3) as opool,
            tc.tile_pool(name="ps", bufs=4, space="PSUM") as pspool,
            tc.tile_pool(name="sc", bufs=4) as scpool,
        ):
            asb = cpool.tile([128, d_out], F32)
            nc.sync.dma_start(out=asb[:], in_=asr[:])
            adb = cpool.tile([128, d_out], F32)
            nc.sync.dma_start(out=adb[:], in_=adr[:])
            bxb = cpool.tile([1, d_out + 2], F16, tag="bx")
            nc.sync.dma_start(out=bxb[:], in_=bex[:])
            one1 = cpool.tile([1, 128], F16, tag="one1")
            nc.vector.memset(one1[:], 1.0)
            esedb = cpool.tile([128, 2 * W], F32)

            xls = []
            for k in range(kc):
                xl = cpool.tile([128, W, 128], F16, tag=f"x{k}")
                nc.sync.dma_start(out=xl[:], in_=xT[k])
                xls.append(xl)

            wsb = []
            for k in range(kc):
                wk = cpool.tile([128, d_out + 2], F16, tag=f"w{k}")
                nc.sync.dma_start(
                    out=wk[:, 0:d_out], in_=Wm[128 * k:128 * (k + 1), :]
                )
                scr = scpool.tile([128, d_out], F32, tag="wes")
                nc.vector.tensor_tensor(
                    out=scr[:], in0=wk[:, 0:d_out], in1=asb[:],
                    op=mybir.AluOpType.mult,
                )
                wes = scpool.tile([128, 1], F32, tag="wesc")
                nc.vector.reduce_sum(
                    out=wes[:], in_=scr[:], axis=mybir.AxisListType.X
                )
                nc.vector.tensor_copy(out=wk[:, d_out:d_out + 1], in_=wes[:])
                scr2 = scpool.tile([128, d_out], F32, tag="wed")
                nc.vector.tensor_tensor(
                    out=scr2[:], in0=wk[:, 0:d_out], in1=adb[:],
                    op=mybir.AluOpType.mult,
                )
                wed = scpool.tile([128, 1], F32, tag="wedc")
                nc.vector.reduce_sum(
                    out=wed[:], in_=scr2[:], axis=mybir.AxisListType.X
                )
                nc.vector.tensor_copy(out=wk[:, d_out + 1:d_out + 2], in_=wed[:])
                wsb.append(wk)

            for wb in range(0, W, WB):
                nb = min(WB, W - wb)
                ob = opool.tile([128, WB, d_out], F16)
                for j in range(nb):
                    w = wb + j
                    ps = pspool.tile([128, d_out + 2], F32, space="PSUM")
                    nc.tensor.matmul(
                        out=ps[:], lhsT=one1[:], rhs=bxb[:],
                        start=True, stop=False,
                    )
                    for k in range(kc):
                        nc.tensor.matmul(
                            out=ps[:], lhsT=xls[k][:, w], rhs=wsb[k][:],
                            start=False, stop=(k == kc - 1),
                        )
                    nc.scalar.activation(
                        out=ob[:, j], in_=ps[:, 0:d_out],
                        func=mybir.ActivationFunctionType.Copy,
                    )
                    nc.vector.tensor_copy(
                        out=esedb[:, 2 * w:2 * w + 2],
                        in_=ps[:, d_out:d_out + 2],
                    )
                nc.sync.dma_start(
                    out=h16[:, wb:wb + nb], in_=ob[:, 0:nb]
                )
            nc.sync.dma_start(out=esed[:], in_=esedb[:])
    nc.compile()
    return nc


def _build_agg(d, T, WT, relu):
    """Aggregation over one GAT layer from host-staged edge tiles.
    gt [128, T*(d+1)] f16 ([H[src]+b | 1] edge rows), eq8 [128, T, 128] fp8
    one-hot dst masks, esx/edx [128, T] f32 -> ho [128, W, d] f16.
    Per window: sp = eq8 * p (one DVE op), WT one-hot matmuls accumulate
    [sum p*h | sum p] in PSUM, epilogue scales by 1/sum p on ACT."""
    D1 = d + 1
    nc = bacc.Bacc(num_devices=NCORES)
    gt = nc.dram_tensor("gt", [128, T * D1], F8, kind="ExternalInput").ap()
    eqm = nc.dram_tensor("eqm", [128, T, 128], F8, kind="ExternalInput").ap()
    esx = nc.dram_tensor("esx", [128, T], F32, kind="ExternalInput").ap()
    edx = nc.dram_tensor("edx", [128, T], F32, kind="ExternalInput").ap()
    ho = nc.dram_tensor("ho", [128, W, d], F16, kind="ExternalOutput").ap()

    CT = CHW * WT                       # tiles per stream chunk
    nchunk = (W + CHW - 1) // CHW
    with tile.TileContext(nc) as tc:
        with (
            tc.tile_pool(name="const", bufs=1) as cpool,
            tc.tile_pool(name="g", bufs=3) as gpool,
            tc.tile_pool(name="e", bufs=3) as epool,
            tc.tile_pool(name="sp", bufs=4) as sppool,
            tc.tile_pool(name="o", bufs=3) as opool,
            tc.tile_pool(name="cl", bufs=4) as clpool,
            tc.tile_pool(name="ps", bufs=4, space="PSUM") as pspool,
        ):
            esxs = cpool.tile([128, T], F32)
            nc.sync.dma_start(out=esxs[:], in_=esx[:])
            edxs = cpool.tile([128, T], F32)
            nc.sync.dma_start(out=edxs[:], in_=edx[:])

            # p = exp(leakyrelu(es+ed, 0.2)) in fp16
            lg = cpool.tile([128, T], F32, tag="lg")
            nc.vector.tensor_tensor(
                out=lg[:], in0=esxs[:], in1=edxs[:], op=mybir.AluOpType.add
            )
            lg2 = cpool.tile([128, T], F32, tag="lg2")
            nc.vector.tensor_scalar_mul(out=lg2[:], in0=lg[:], scalar1=0.2)
            nc.vector.tensor_tensor(
                out=lg[:], in0=lg[:], in1=lg2[:], op=mybir.AluOpType.max
            )
            p16 = cpool.tile([128, T], F16, tag="p16")
            nc.scalar.activation(
                out=p16[:], in_=lg[:], func=mybir.ActivationFunctionType.Exp
            )

            gts, eqs = [], []
            for ci in range(nchunk):
                c0, c1 = ci * CT, min((ci + 1) * CT, T)
                gtile = gpool.tile([128, (c1 - c0) * D1], F8)
                nc.sync.dma_start(out=gtile[:], in_=gt[:, c0 * D1:c1 * D1])
                gts.append((gtile, c0))
                etile = epool.tile([128, c1 - c0, 128], F8)
                nc.sync.dma_start(out=etile[:], in_=eqm[:, c0:c1])
                eqs.append((etile, c0))

            # one-hot mults run over SPG-window groups (chunk-aligned since
            # CHW % SPG == 0 or SPG % CHW == 0 keeps groups within... groups
            # must not span stream chunks: CHW == SPG ensures alignment.
            sps = {}
            for wb in range(0, W, WB):
                nb = min(WB, W - wb)
                ob = opool.tile([128, WB, d], F16)
                for j in range(nb):
                    w = wb + j
                    t0 = w * WT
                    if w % SPG == 0:
                        ng = min(SPG, W - w)
                        # sp[p, t, q] = eq8[p, t0+t, q] * p16[p, t0+t]
                        sp3 = sppool.tile([128, SPG * WT, 128], F16)
                        etile, e0 = eqs[(t0 // CT)]
                        p_b = p16[:, t0:t0 + ng * WT].unsqueeze(2).to_broadcast(
                            [128, ng * WT, 128])
                        nc.vector.tensor_tensor(
                            out=sp3[:, 0:ng * WT],
                            in0=etile[:, t0 - e0:t0 - e0 + ng * WT],
                            in1=p_b, op=mybir.AluOpType.mult,
                        )
                        sps[w] = sp3
                    sp3 = sps[w - w % SPG]
                    toff = (w % SPG) * WT
                    ps = pspool.tile([128, D1], F32, space="PSUM")
                    for t in range(WT):
                        gidx = t0 + t
                        gtile, c0 = gts[gidx // CT]
                        rhs = gtile[:, (gidx - c0) * D1:(gidx - c0 + 1) * D1]
                        nc.tensor.matmul(
                            out=ps[:], lhsT=sp3[:, toff + t], rhs=rhs,
                            start=(t == 0), stop=(t == WT - 1),
                        )
                    rec = clpool.tile([128, 1], F32)
                    nc.vector.reciprocal(rec[:], ps[:, d:D1])
                    nc.scalar.activation(
                        out=ob[:, j], in_=ps[:, 0:d],
                        func=(mybir.ActivationFunctionType.Relu if relu
                              else mybir.ActivationFunctionType.Copy),
                        scale=rec[:, :1],
                    )
                nc.sync.dma_start(out=ho[:, wb:wb + nb], in_=ob[:, 0:nb])
    nc.compile()
    return nc


def _build_link(pt):
    """Link predictor from host-staged row tiles:
    z = sigmoid(sum_f g0*wl0 + sum_f g1*wl1 + bl) for pt*128 pairs."""
    nc = bacc.Bacc(num_devices=NCORES)
    g0 = nc.dram_tensor("g0", [128, pt * F_IN], F16, kind="ExternalInput").ap()
    g1 = nc.dram_tensor("g1", [128, pt * F_IN], F16, kind="ExternalInput").ap()
    wl0 = nc.dram_tensor("wl0", [128, F_IN], F32, kind="ExternalInput").ap()
    wl1 = nc.dram_tensor("wl1", [128, F_IN], F32, kind="ExternalInput").ap()
    blr = nc.dram_tensor("blr", [128, 1], F32, kind="ExternalInput").ap()
    z = nc.dram_tensor("z", [128, pt], F32, kind="ExternalOutput").ap()

    with tile.TileContext(nc) as tc:
        with (
            tc.tile_pool(name="const", bufs=1) as cpool,
            tc.tile_pool(name="sc", bufs=6) as scpool,
        ):
            w0s = cpool.tile([128, F_IN], F32)
            nc.sync.dma_start(out=w0s[:], in_=wl0[:])
            w1s = cpool.tile([128, F_IN], F32)
            nc.sync.dma_start(out=w1s[:], in_=wl1[:])
            bls = cpool.tile([128, 1], F32)
            nc.sync.dma_start(out=bls[:], in_=blr[:])
            zsb = cpool.tile([128, pt], F32)

            g0s = cpool.tile([128, pt * F_IN], F16, tag="g0s")
            nc.sync.dma_start(out=g0s[:], in_=g0[:])
            g1s = cpool.tile([128, pt * F_IN], F16, tag="g1s")
            nc.sync.dma_start(out=g1s[:], in_=g1[:])

            for t in range(pt):
                scr = scpool.tile([128, F_IN], F32, tag="scr0")
                s0 = scpool.tile([128, 1], F32, tag="s0")
                nc.vector.scalar_tensor_tensor(
                    out=scr[:], in0=g0s[:, t * F_IN:(t + 1) * F_IN],
                    scalar=1.0, in1=w0s[:],
                    op0=mybir.AluOpType.mult, op1=mybir.AluOpType.mult,
                    accum_out=s0[:],
                )
                scr1 = scpool.tile([128, F_IN], F32, tag="scr1")
                s1 = scpool.tile([128, 1], F32, tag="s1")
                nc.vector.scalar_tensor_tensor(
                    out=scr1[:], in0=g1s[:, t * F_IN:(t + 1) * F_IN],
                    scalar=1.0, in1=w1s[:],
                    op0=mybir.AluOpType.mult, op1=mybir.AluOpType.mult,
                    accum_out=s1[:],
                )
                ssum = scpool.tile([128, 1], F32, tag="ss")
                nc.vector.tensor_tensor(
                    out=ssum[:], in0=s0[:], in1=s1[:], op=mybir.AluOpType.add
                )
                nc.scalar.activation(
                    out=zsb[:, t:t + 1], in_=ssum[:],
                    func=mybir.ActivationFunctionType.Sigmoid, bias=bls[:, :1],
                )
            nc.sync.dma_start(out=z[:], in_=zsb[:])
    nc.compile()
    return nc


def _run(name, nc, in_maps, trace=True):
    last = None
    for attempt in range(3):
        try:
            res = run_bass_kernel_spmd(
                nc, in_maps, core_ids=list(range(NCORES)), trace=trace
            )
            LAST_EXEC_NS[name] = res.exec_time_ns
            return res.results
        except Exception as e:  # wedged-device retry (clears on re-attempt)
            last = e
            time.sleep(5)
    raise last


def _rep(v, n=128):
    return np.ascontiguousarray(np.broadcast_to(np.asarray(v, np.float32), (n, len(v))))


def _tile_xT(xfull_shards, kc, d_in):
    """list of [NSP, d_in] fp16 per core -> [NCORES, kc, 128, W, 128] fp16
    (partition-major: xT[c,k,p,w,f] = x[w*128+f? no: x^T tiles)."""
    out = np.zeros((NCORES, kc, 128, W, 128), np.float16)
    for c in range(NCORES):
        xt = xfull_shards[c].T  # [d_in, NSP]
        for k in range(kc):
            blk = xt[128 * k:128 * (k + 1)].reshape(128, W, 128)
            out[c, k] = blk
    return out


# ------------------------------------------------------------------- kernel
def kernel(features, edge_index, mask, W1, a_src1, a_dst1, b1, W2, a_src2,
           a_dst2, b2, Wl, bl):
    features = np.asarray(features, np.float32)
    edge_index = np.asarray(edge_index, np.int32)
    mask = np.asarray(mask, np.int32)
    W1, W2, Wl = (np.asarray(a, np.float32) for a in (W1, W2, Wl))
    a_src1, a_dst1, b1 = (np.asarray(a, np.float32) for a in (a_src1, a_dst1, b1))
    a_src2, a_dst2, b2 = (np.asarray(a, np.float32) for a in (a_src2, a_dst2, b2))
    bl = np.asarray(bl, np.float32)

    g = _prep_graph(edge_index)
    T, WT = g["T"], g["WT"]

    key = (T, WT)
    if key not in _PROG_CACHE:
        _PROG_CACHE[key] = dict(
            p1=_build_proj(1, H),
            a1=_build_agg(H, T, WT, relu=True),
            p2=_build_proj(2, F_IN),
            a2=_build_agg(F_IN, T, WT, relu=False),
            lk=_build_link((10000 // NCORES + 127) // 128),
        )
    progs = _PROG_CACHE[key]

    # ---- L1: H1 = X @ W1 (sharded, window-permuted rows), es1/ed1
    xsh = []
    for c in range(NCORES):
        xs = np.zeros((NSP, F_IN), np.float16)
        r2n = g["row2node"][c]
        m = r2n >= 0
        xs[m] = features[r2n[m]]
        xsh.append(xs)
    xT1 = _tile_xT(xsh, 1, F_IN)
    W1h = W1.astype(np.float16)
    bex1 = np.concatenate([b1, [0.0, 0.0]]).astype(np.float16)[None, :]
    r1 = _run("p1", progs["p1"], [
        dict(xT=xT1[c], Wm=W1h, asr=_rep(a_src1), adr=_rep(a_dst1), bex=bex1)
        for c in range(NCORES)
    ])
    H1e = _full_from_shards([r1[c]["h16"] for c in range(NCORES)], g, H)
    esed1 = _full_from_shards(
        [r1[c]["esed"].reshape(128, W, 2) for c in range(NCORES)], g, 2)
    es1, ed1 = esed1[:, 0], esed1[:, 1]

    # ---- L2: aggregate layer 1 -> h1r = relu(agg) (b1 already in table rows)
    ins2 = []
    for c in range(NCORES):
        esx, edx = _expand(es1, ed1, g, c)
        ins2.append(dict(gt=_gt_tiles(H1e, g, c), eqm=g["eq8"][c],
                         esx=esx, edx=edx))
    r2 = _run("a1", progs["a1"], ins2)
    h1r = [np.ascontiguousarray(r2[c]["ho"].transpose(1, 0, 2)).reshape(NSP, H)
           for c in range(NCORES)]

    # ---- L3: H2 = h1r @ W2, es2/ed2
    xT2 = _tile_xT(h1r, 2, H)
    W2h = W2.astype(np.float16)
    bex2 = np.concatenate([b2, [0.0, 0.0]]).astype(np.float16)[None, :]
    r3 = _run("p2", progs["p2"], [
        dict(xT=xT2[c], Wm=W2h, asr=_rep(a_src2), adr=_rep(a_dst2), bex=bex2)
        for c in range(NCORES)
    ])
    H2e = _full_from_shards([r3[c]["h16"] for c in range(NCORES)], g, F_IN)
    esed2 = _full_from_shards(
        [r3[c]["esed"].reshape(128, W, 2) for c in range(NCORES)], g, 2)
    es2, ed2 = esed2[:, 0], esed2[:, 1]

    # ---- L4: aggregate layer 2 -> h2 = agg (b2 already in table rows)
    ins4 = []
    for c in range(NCORES):
        esx, edx = _expand(es2, ed2, g, c)
        ins4.append(dict(gt=_gt_tiles(H2e, g, c), eqm=g["eq8"][c],
                         esx=esx, edx=edx))
    r4 = _run("a2", progs["a2"], ins4)
    H2f = _full_from_shards([r4[c]["ho"] for c in range(NCORES)], g, F_IN)

    # ---- L5: link predictor (host-staged row tiles)
    P = mask.shape[0]
    pc = P // NCORES
    pt = (pc + 127) // 128
    mT = mask.T
    wl0 = _rep(Wl[:F_IN, 0])
    wl1 = _rep(Wl[F_IN:, 0])
    blr = np.full((128, 1), float(bl[0]), np.float32)
    ins5 = []
    for c in range(NCORES):
        m0 = np.zeros((128, pt), np.int64)
        m1 = np.zeros((128, pt), np.int64)
        s = np.arange(pc)
        m0[s % 128, s // 128] = mT[0][c * pc:(c + 1) * pc]
        m1[s % 128, s // 128] = mT[1][c * pc:(c + 1) * pc]
        g0 = H2f[m0].reshape(128, pt * F_IN)
        g1 = H2f[m1].reshape(128, pt * F_IN)
        ins5.append(dict(g0=g0, g1=g1, wl0=wl0, wl1=wl1, blr=blr))
    r5 = _run("lk", progs["lk"], ins5)
    out = np.zeros((P, 1), np.float32)
    for c in range(NCORES):
        s = np.arange(pc)
        out[c * pc:(c + 1) * pc, 0] = r5[c]["z"][s % 128, s // 128]

    tot = sum(v for v in LAST_EXEC_NS.values() if v)
    print(f"kernel launches ns: {LAST_EXEC_NS} total {tot}")
    return out


# revision 28
# speedup vs baseline: 1.4218x; 1.2925x over previous
"""Two-layer GAT (single-head, PyG-style) + link predictor on 8 TRN2 NeuronCores.

Strategy (memory-regime):
  - Nodes sharded 8-way by id (6250/core, padded to 6272 = 49 windows of 128).
    Within a core, nodes are packed into windows by (in-degree+1) greedy
    bin-packing so every window holds <= 128 nodes and <= 128*WT edge slots;
    all windows share a uniform tile count WT (slot-major [128, T] layout).
  - Self-loops are ordinary edge slots (src == dst). Edge softmax needs no
    max-shift (the shift cancels in the ratio; logits are O(10)).
  - Halo exchange runs between launches on the host as pure index-space
    movement: per-edge source-feature tiles gt[p,t,:] = [H[src[p,t]] | 1.0]
    and per-edge es/ed scalars are assembled with numpy fancy indexing and
    staged as kernel inputs; the device streams them with large sequential
    DMAs (no indirect gathers). All floating-point math (projections,
    exp/leaky-relu, softmax-weighted scatter via one-hot matmuls,
    normalization, link predictor) happens on device.
  - The trailing 1.0 column of every edge row makes the same one-hot matmul
    accumulate the softmax denominator:
        ps[dst, 0:d] += sum_e p_e [dstrow_e == dst] H[src_e]
        ps[dst, d]   += sum_e p_e [dstrow_e == dst]
    One-hot tiles are built per-window in one batched DVE/Pool op pair using
    stride-0 broadcast access patterns; a per-window epilogue normalizes,
    adds bias (and relu for layer 1).
  - Dense projections run sharded on PE in fp16; es = h@a_s / ed = h@a_d come
    free as two extra matmul columns [W | W@a_s | W@a_d].

Launches: L1 proj1 -> L2 agg1 -> L3 proj2 -> L4 agg2 -> L5 link predictor.
"""
import heapq
import time

import numpy as np

import concourse.bass as bass
import concourse.mybir as mybir
import concourse.tile as tile
from concourse import bacc
from concourse.bass_utils import run_bass_kernel_spmd

F32 = mybir.dt.float32
F16 = mybir.dt.float16
F8 = mybir.dt.float8e4
I32 = mybir.dt.int32

NCORES = 8
N, F_IN, H, C = 50000, 128, 256, 1
NS = N // NCORES            # 6250 nodes per shard
W = (NS + 127) // 128       # 49 windows per shard
NSP = W * 128               # 6272 padded slots
NEG = -1.0e30               # pad-edge sentinel (exp -> exactly 0)
CHW = 3                     # windows per streaming DMA chunk
SPG = 3                     # windows per batched one-hot mult
WB = 7                      # windows per batched output write (49 = 7*7)

LAST_EXEC_NS = {}           # launch name -> exec_time_ns (filled per kernel() call)
_PROG_CACHE = {}


# ----------------------------------------------------------------- host prep
def _prep_graph(edge_index):
    """Per core: pack nodes into 49 windows by (deg+1) so all windows fit in
    <=128 nodes and a uniform WT tiles of 128 edge slots; lay self-loop +
    incoming edges of each window into slot-major [128, T] layout."""
    src = np.asarray(edge_index[0], np.int64)
    dst = np.asarray(edge_index[1], np.int64)
    deg = np.bincount(dst, minlength=N)

    order = np.argsort(dst, kind="stable")
    src_s = src[order]
    estart = np.concatenate([[0], np.cumsum(deg)])

    win_nodes = np.full((NCORES, W, 128), -1, np.int64)
    win_count = np.zeros((NCORES, W), np.int64)
    win_load = np.zeros((NCORES, W), np.int64)
    for c in range(NCORES):
        nodes = np.arange(c * NS, (c + 1) * NS)
        wgt = deg[nodes] + 1
        ordn = np.argsort(-wgt, kind="stable")
        heap = [(0, w) for w in range(W)]
        heapq.heapify(heap)
        skipped = []
        for i in ordn:
            n, gw = nodes[i], wgt[i]
            while True:
                load, w = heapq.heappop(heap)
                if win_count[c, w] < 128:
                    break
                skipped.append((load, w))
            for it in skipped:
                heapq.heappush(heap, it)
            skipped = []
            win_nodes[c, w, win_count[c, w]] = n
            win_count[c, w] += 1
            win_load[c, w] = load + gw
            heapq.heappush(heap, (load + gw, w))
    WT = max(int(np.ceil(win_load.max() / 128)), 1)
    T = W * WT

    srcs = np.zeros((NCORES, 128, T), np.int32)
    srcg = np.zeros((NCORES, 128, T), np.int64)
    dstg = np.zeros((NCORES, 128, T), np.int64)
    dstr = np.full((NCORES, 128, T), 128, np.int64)   # local dst row, 128 = pad
    pad = np.ones((NCORES, 128, T), bool)
    row2node = np.full((NCORES, NSP), -1, np.int64)

    for c in range(NCORES):
        for w in range(W):
            cnt = int(win_count[c, w])
            nl = win_nodes[c, w, :cnt]
            row2node[c, w * 128:w * 128 + cnt] = nl
            seg_src, seg_row = [], []
            for r, n in enumerate(nl):
                e0, e1 = int(estart[n]), int(estart[n + 1])
                ss = np.concatenate([[n], src_s[e0:e1]])
                seg_src.append(ss)
                seg_row.append(np.full(len(ss), r, np.int64))
            ss = np.concatenate(seg_src)
            rr = np.concatenate(seg_row)
            sl = np.arange(len(ss))
            pp, tt = sl % 128, w * WT + sl // 128
            srcs[c, pp, tt] = ss
            srcg[c, pp, tt] = ss
            dstg[c, pp, tt] = nl[rr]
            dstr[c, pp, tt] = rr
            pad[c, pp, tt] = False
    # fp8 one-hot mask (row 128 of eyeZ = zeros for pads); shared by both layers
    f8np = mybir.dt.np(F8)
    eyeZ = np.zeros((129, 128), f8np)
    eyeZ[np.arange(128), np.arange(128)] = 1.0
    eq8 = eyeZ[dstr]                                   # [NCORES, 128, T, 128]
    return dict(srcs=srcs, srcg=srcg, dstg=dstg, eq8=eq8, pad=pad,
                row2node=row2node, WT=WT, T=T)


def _expand(es_full, ed_full, g, c):
    esx = es_full[g["srcg"][c]].astype(np.float32)
    edx = ed_full[g["dstg"][c]].astype(np.float32)
    p = g["pad"][c]
    esx[p] = NEG
    edx[p] = 0.0
    return esx, edx


def _full_from_shards(shards, g, cols):
    """shards: per-core [128, W, cols] (row w*128+p) -> node-indexed [N, cols]."""
    out = np.zeros((N, cols), shards[0].dtype)
    for c in range(NCORES):
        flat = np.ascontiguousarray(shards[c].transpose(1, 0, 2)).reshape(NSP, cols)
        r2n = g["row2node"][c]
        m = r2n >= 0
        out[r2n[m]] = flat[m]
    return out


def _gt_tiles(Hfull, g, c):
    """[128, T*(d+1)] fp8 edge tiles [H[src] | 1.0] (1.0 exact in e4m3)."""
    d = Hfull.shape[1]
    gt = np.ones((128, g["T"], d + 1), mybir.dt.np(F8))
    gt[:, :, :d] = Hfull[g["srcs"][c]].astype(mybir.dt.np(F8))
    return gt.reshape(128, g["T"] * (d + 1))


# ------------------------------------------------------------- bass programs
def _build_proj(kc, d_out):
    """Projection: psum = bias_ext + x @ [W | W@a_s | W@a_d] per 128-node
    window. The layer bias rides in via a rank-1 ones-row matmul (softmax
    weights sum to 1, so adding b to every table row equals adding b after
    aggregation); its es/ed columns are zero so the attention dots stay
    bias-free. xT fp16 [kc, 128, W, 128], Wm fp16 [kc*128, d_out],
    asr/adr fp32 [128, d_out], bex fp16 [1, d_out+2] = [b | 0 0].
    Outputs h16 [128, W, d_out] fp16, esed [128, 2W] f32 (interleaved es,ed)."""
    nc = bacc.Bacc(num_devices=NCORES)
    xT = nc.dram_tensor("xT", [kc, 128, W, 128], F16, kind="ExternalInput").ap()
    Wm = nc.dram_tensor("Wm", [kc * 128, d_out], F16, kind="ExternalInput").ap()
    asr = nc.dram_tensor("asr", [128, d_out], F32, kind="ExternalInput").ap()
    adr = nc.dram_tensor("adr", [128, d_out], F32, kind="ExternalInput").ap()
    bex = nc.dram_tensor("bex", [1, d_out + 2], F16, kind="ExternalInput").ap()
    h16 = nc.dram_tensor("h16", [128, W, d_out], F16, kind="ExternalOutput").ap()
    esed = nc.dram_tensor("esed", [128, 2 * W], F32, kind="ExternalOutput").ap()

    with tile.TileContext(nc) as tc:
        with (
            tc.tile_pool(name="const", bufs=1) as cpool,
            tc.tile_pool(name="o", bufs=3) as opool,
            tc.tile_pool(name="ps", bufs=4, space="PSUM") as pspool,
            tc.tile_pool(name="sc", bufs=4) as scpool,
        ):
            asb = cpool.tile([128, d_out], F32)
            nc.sync.dma_start(out=asb[:], in_=asr[:])
            adb = cpool.tile([128, d_out], F32)
            nc.sync.dma_start(out=adb[:], in_=adr[:])
            bxb = cpool.tile([1, d_out + 2], F16, tag="bx")
            nc.sync.dma_start(out=bxb[:], in_=bex[:])
            one1 = cpool.tile([1, 128], F16, tag="one1")
            nc.vector.memset(one1[:], 1.0)
            esedb = cpool.tile([128, 2 * W], F32)

            xls = []
            for k in range(kc):
                xl = cpool.tile([128, W, 128], F16, tag=f"x{k}")
                nc.sync.dma_start(out=xl[:], in_=xT[k])
                xls.append(xl)

            wsb = []
            for k in range(kc):
                wk = cpool.tile([128, d_out + 2], F16, tag=f"w{k}")
                nc.sync.dma_start(
                    out=wk[:, 0:d_out], in_=Wm[128 * k:128 * (k + 1), :]
                )
                scr = scpool.tile([128, d_out], F32, tag="wes")
                nc.vector.tensor_tensor(
                    out=scr[:], in0=wk[:, 0:d_out], in1=asb[:],
                    op=mybir.AluOpType.mult,
                )
                wes = scpool.tile([128, 1], F32, tag="wesc")
                nc.vector.reduce_sum(
                    out=wes[:], in_=scr[:], axis=mybir.AxisListType.X
                )
                nc.vector.tensor_copy(out=wk[:, d_out:d_out + 1], in_=wes[:])
                scr2 = scpool.tile([128, d_out], F32, tag="wed")
                nc.vector.tensor_tensor(
                    out=scr2[:], in0=wk[:, 0:d_out], in1=adb[:],
                    op=mybir.AluOpType.mult,
                )
                wed = scpool.tile([128, 1], F32, tag="wedc")
                nc.vector.reduce_sum(
                    out=wed[:], in_=scr2[:], axis=mybir.AxisListType.X
                )
                nc.vector.tensor_copy(out=wk[:, d_out + 1:d_out + 2], in_=wed[:])
                wsb.append(wk)

            for wb in range(0, W, WB):
                nb = min(WB, W - wb)
                ob = opool.tile([128, WB, d_out], F16)
                for j in range(nb):
                    w = wb + j
                    ps = pspool.tile([128, d_out + 2], F32, space="PSUM")
                    nc.tensor.matmul(
                        out=ps[:], lhsT=one1[:], rhs=bxb[:],
                        start=True, stop=False,
                    )
                    for k in range(kc):
                        nc.tensor.matmul(
                            out=ps[:], lhsT=xls[k][:, w], rhs=wsb[k][:],
                            start=False, stop=(k == kc - 1),
                        )
                    nc.scalar.activation(
                        out=ob[:, j], in_=ps[:, 0:d_out],
                        func=mybir.ActivationFunctionType.Copy,
                    )
                    nc.vector.tensor_copy(
                        out=esedb[:, 2 * w:2 * w + 2],
                        in_=ps[:, d_out:d_out + 2],
                    )
                nc.sync.dma_start(
                    out=h16[:, wb:wb + nb], in_=ob[:, 0:nb]
                )
            nc.sync.dma_start(out=esed[:], in_=esedb[:])
    nc.compile()
    return nc


def _build_agg(d, T, WT, relu, proj_d2=None):
    """Aggregation over one GAT layer from host-staged edge tiles.
    gt [128, T*(d+1)] fp8 ([H[src]+b | 1] edge rows), eq8 [128, T, 128] fp8
    one-hot dst masks, esx/edx [128, T] f32 -> ho [128, W, d] f16.
    Per window: sp = eq8 * p (one DVE op), WT one-hot matmuls accumulate
    [sum p*h | sum p] in PSUM, epilogue scales by 1/sum p on ACT.
    With proj_d2 set, the next GAT layer's projection is fused per window:
    h1r is PE-transposed, h2 = bias2 + h1r @ [W2 | W2@a_s2 | W2@a_d2], and
    h2o [128, W, proj_d2] f16 + esed2 [128, 2W] f32 are extra outputs."""
    D1 = d + 1
    nc = bacc.Bacc(num_devices=NCORES)
    gt = nc.dram_tensor("gt", [128, T * D1], F8, kind="ExternalInput").ap()
    eqm = nc.dram_tensor("eqm", [128, T, 128], F8, kind="ExternalInput").ap()
    esx = nc.dram_tensor("esx", [128, T], F32, kind="ExternalInput").ap()
    edx = nc.dram_tensor("edx", [128, T], F32, kind="ExternalInput").ap()
    ho = nc.dram_tensor("ho", [128, W, d], F16, kind="ExternalOutput").ap()
    if proj_d2 is not None:
        kc2 = d // 128
        Wm2 = nc.dram_tensor("Wm2", [d, proj_d2], F16, kind="ExternalInput").ap()
        asr2 = nc.dram_tensor("asr2", [128, proj_d2], F32, kind="ExternalInput").ap()
        adr2 = nc.dram_tensor("adr2", [128, proj_d2], F32, kind="ExternalInput").ap()
        bex2 = nc.dram_tensor("bex2", [1, proj_d2 + 2], F16, kind="ExternalInput").ap()
        idt = nc.dram_tensor("idt", [128, 128], F16, kind="ExternalInput").ap()
        h2o = nc.dram_tensor("h2o", [128, W, proj_d2], F16, kind="ExternalOutput").ap()
        esed2 = nc.dram_tensor("esed2", [128, 2 * W], F32, kind="ExternalOutput").ap()

    CT = CHW * WT                       # tiles per stream chunk
    nchunk = (W + CHW - 1) // CHW
    with tile.TileContext(nc) as tc:
        with (
            tc.tile_pool(name="const", bufs=1) as cpool,
            tc.tile_pool(name="g", bufs=3) as gpool,
            tc.tile_pool(name="e", bufs=3) as epool,
            tc.tile_pool(name="sp", bufs=4) as sppool,
            tc.tile_pool(name="o", bufs=3) as opool,
            tc.tile_pool(name="cl", bufs=4) as clpool,
            tc.tile_pool(name="ps", bufs=3, space="PSUM") as pspool,
            tc.tile_pool(name="pt", bufs=2, space="PSUM") as ptpool,
            tc.tile_pool(name="p2", bufs=2, space="PSUM") as p2pool,
            tc.tile_pool(name="ht", bufs=3) as htpool,
            tc.tile_pool(name="o2", bufs=3) as o2pool,
            tc.tile_pool(name="sc", bufs=4) as scpool,
        ):
            esxs = cpool.tile([128, T], F32)
            nc.sync.dma_start(out=esxs[:], in_=esx[:])
            edxs = cpool.tile([128, T], F32)
            nc.sync.dma_start(out=edxs[:], in_=edx[:])

            if proj_d2 is not None:
                as2b = cpool.tile([128, proj_d2], F32, tag="as2")
                nc.sync.dma_start(out=as2b[:], in_=asr2[:])
                ad2b = cpool.tile([128, proj_d2], F32, tag="ad2")
                nc.sync.dma_start(out=ad2b[:], in_=adr2[:])
                bx2b = cpool.tile([1, proj_d2 + 2], F16, tag="bx2")
                nc.sync.dma_start(out=bx2b[:], in_=bex2[:])
                idtb = cpool.tile([128, 128], F16, tag="idt")
                nc.sync.dma_start(out=idtb[:], in_=idt[:])
                one1 = cpool.tile([1, 128], F16, tag="one1")
                nc.vector.memset(one1[:], 1.0)
                esed2b = cpool.tile([128, 2 * W], F32, tag="esed2b")
                w2sb = []
                for k in range(kc2):
                    wk = cpool.tile([128, proj_d2 + 2], F16, tag=f"w2{k}")
                    nc.sync.dma_start(
                        out=wk[:, 0:proj_d2], in_=Wm2[128 * k:128 * (k + 1), :]
                    )
                    scr = scpool.tile([128, proj_d2], F32, tag="wes")
                    nc.vector.tensor_tensor(
                        out=scr[:], in0=wk[:, 0:proj_d2], in1=as2b[:],
                        op=mybir.AluOpType.mult,
                    )
                    wes = scpool.tile([128, 1], F32, tag="wesc")
                    nc.vector.reduce_sum(
                        out=wes[:], in_=scr[:], axis=mybir.AxisListType.X
                    )
                    nc.vector.tensor_copy(
                        out=wk[:, proj_d2:proj_d2 + 1], in_=wes[:])
                    scr2 = scpool.tile([128, proj_d2], F32, tag="wed")
                    nc.vector.tensor_tensor(
                        out=scr2[:], in0=wk[:, 0:proj_d2], in1=ad2b[:],
                        op=mybir.AluOpType.mult,
                    )
                    wed = scpool.tile([128, 1], F32, tag="wedc")
                    nc.vector.reduce_sum(
                        out=wed[:], in_=scr2[:], axis=mybir.AxisListType.X
                    )
                    nc.vector.tensor_copy(
                        out=wk[:, proj_d2 + 1:proj_d2 + 2], in_=wed[:])
                    w2sb.append(wk)

            # p = exp(leakyrelu(es+ed, 0.2)) in fp16
            lg = cpool.tile([128, T], F32, tag="lg")
            nc.vector.tensor_tensor(
                out=lg[:], in0=esxs[:], in1=edxs[:], op=mybir.AluOpType.add
            )
            lg2 = cpool.tile([128, T], F32, tag="lg2")
            nc.vector.tensor_scalar_mul(out=lg2[:], in0=lg[:], scalar1=0.2)
            nc.vector.tensor_tensor(
                out=lg[:], in0=lg[:], in1=lg2[:], op=mybir.AluOpType.max
            )
            p16 = cpool.tile([128, T], F16, tag="p16")
            nc.scalar.activation(
                out=p16[:], in_=lg[:], func=mybir.ActivationFunctionType.Exp
            )

            gts, eqs = [], []
            for ci in range(nchunk):
                c0, c1 = ci * CT, min((ci + 1) * CT, T)
                gtile = gpool.tile([128, (c1 - c0) * D1], F8)
                nc.sync.dma_start(out=gtile[:], in_=gt[:, c0 * D1:c1 * D1])
                gts.append((gtile, c0))
                etile = epool.tile([128, c1 - c0, 128], F8)
                nc.sync.dma_start(out=etile[:], in_=eqm[:, c0:c1])
                eqs.append((etile, c0))

            # one-hot mults run over SPG-window groups (chunk-aligned since
            # CHW % SPG == 0 or SPG % CHW == 0 keeps groups within... groups
            # must not span stream chunks: CHW == SPG ensures alignment.
            sps = {}
            for wb in range(0, W, WB):
                nb = min(WB, W - wb)
                ob = opool.tile([128, WB, d], F16)
                if proj_d2 is not None:
                    ob2 = o2pool.tile([128, WB, proj_d2], F16)
                for j in range(nb):
                    w = wb + j
                    t0 = w * WT
                    if w % SPG == 0:
                        ng = min(SPG, W - w)
                        # sp[p, t, q] = eq8[p, t0+t, q] * p16[p, t0+t]
                        sp3 = sppool.tile([128, SPG * WT, 128], F16)
                        etile, e0 = eqs[(t0 // CT)]
                        p_b = p16[:, t0:t0 + ng * WT].unsqueeze(2).to_broadcast(
                            [128, ng * WT, 128])
                        nc.vector.tensor_tensor(
                            out=sp3[:, 0:ng * WT],
                            in0=etile[:, t0 - e0:t0 - e0 + ng * WT],
                            in1=p_b, op=mybir.AluOpType.mult,
                        )
                        sps[w] = sp3
                    sp3 = sps[w - w % SPG]
                    toff = (w % SPG) * WT
                    ps = pspool.tile([128, D1], F32, space="PSUM")
                    for t in range(WT):
                        gidx = t0 + t
                        gtile, c0 = gts[gidx // CT]
                        rhs = gtile[:, (gidx - c0) * D1:(gidx - c0 + 1) * D1]
                        nc.tensor.matmul(
                            out=ps[:], lhsT=sp3[:, toff + t], rhs=rhs,
                            start=(t == 0), stop=(t == WT - 1),
                        )
                    rec = clpool.tile([128, 1], F32)
                    nc.vector.reciprocal(rec[:], ps[:, d:D1])
                    nc.scalar.activation(
                        out=ob[:, j], in_=ps[:, 0:d],
                        func=(mybir.ActivationFunctionType.Relu if relu
                              else mybir.ActivationFunctionType.Copy),
                        scale=rec[:, :1],
                    )
                    if proj_d2 is not None:
                        # fused next-layer projection on the fresh h1r window
                        ht = htpool.tile([128, kc2, 128], F16)
                        for k in range(kc2):
                            psT = ptpool.tile([128, 128], F16, space="PSUM")
                            nc.tensor.transpose(
                                psT[:], ob[:, j][:, 128 * k:128 * (k + 1)],
                                idtb[:],
                            )
                            nc.scalar.activation(
                                out=ht[:, k], in_=psT[:],
                                func=mybir.ActivationFunctionType.Copy,
                            )
                        ps2 = p2pool.tile([128, proj_d2 + 2], F32, space="PSUM")
                        nc.tensor.matmul(
                            out=ps2[:], lhsT=one1[:], rhs=bx2b[:],
                            start=True, stop=False,
                        )
                        for k in range(kc2):
                            nc.tensor.matmul(
                                out=ps2[:], lhsT=ht[:, k], rhs=w2sb[k][:],
                                start=False, stop=(k == kc2 - 1),
                            )
                        nc.scalar.activation(
                            out=ob2[:, j], in_=ps2[:, 0:proj_d2],
                            func=mybir.ActivationFunctionType.Copy,
                        )
                        nc.vector.tensor_copy(
                            out=esed2b[:, 2 * w:2 * w + 2],
                            in_=ps2[:, proj_d2:proj_d2 + 2],
                        )
                nc.sync.dma_start(out=ho[:, wb:wb + nb], in_=ob[:, 0:nb])
                if proj_d2 is not None:
                    nc.sync.dma_start(out=h2o[:, wb:wb + nb], in_=ob2[:, 0:nb])
            if proj_d2 is not None:
                nc.sync.dma_start(out=esed2[:], in_=esed2b[:])
    nc.compile()
    return nc


def _build_link(pt):
    """Link predictor from host-staged row tiles:
    z = sigmoid(sum_f g0*wl0 + sum_f g1*wl1 + bl) for pt*128 pairs."""
    nc = bacc.Bacc(num_devices=NCORES)
    g0 = nc.dram_tensor("g0", [128, pt * F_IN], F16, kind="ExternalInput").ap()
    g1 = nc.dram_tensor("g1", [128, pt * F_IN], F16, kind="ExternalInput").ap()
    wl0 = nc.dram_tensor("wl0", [128, F_IN], F32, kind="ExternalInput").ap()
    wl1 = nc.dram_tensor("wl1", [128, F_IN], F32, kind="ExternalInput").ap()
    blr = nc.dram_tensor("blr", [128, 1], F32, kind="ExternalInput").ap()
    z = nc.dram_tensor("z", [128, pt], F32, kind="ExternalOutput").ap()

    with tile.TileContext(nc) as tc:
        with (
            tc.tile_pool(name="const", bufs=1) as cpool,
            tc.tile_pool(name="sc", bufs=6) as scpool,
        ):
            w0s = cpool.tile([128, F_IN], F32)
            nc.sync.dma_start(out=w0s[:], in_=wl0[:])
            w1s = cpool.tile([128, F_IN], F32)
            nc.sync.dma_start(out=w1s[:], in_=wl1[:])
            bls = cpool.tile([128, 1], F32)
            nc.sync.dma_start(out=bls[:], in_=blr[:])
            zsb = cpool.tile([128, pt], F32)

            g0s = cpool.tile([128, pt * F_IN], F16, tag="g0s")
            nc.sync.dma_start(out=g0s[:], in_=g0[:])
            g1s = cpool.tile([128, pt * F_IN], F16, tag="g1s")
            nc.sync.dma_start(out=g1s[:], in_=g1[:])

            for t in range(pt):
                scr = scpool.tile([128, F_IN], F32, tag="scr0")
                s0 = scpool.tile([128, 1], F32, tag="s0")
                nc.vector.scalar_tensor_tensor(
                    out=scr[:], in0=g0s[:, t * F_IN:(t + 1) * F_IN],
                    scalar=1.0, in1=w0s[:],
                    op0=mybir.AluOpType.mult, op1=mybir.AluOpType.mult,
                    accum_out=s0[:],
                )
                scr1 = scpool.tile([128, F_IN], F32, tag="scr1")
                s1 = scpool.tile([128, 1], F32, tag="s1")
                nc.vector.scalar_tensor_tensor(
                    out=scr1[:], in0=g1s[:, t * F_IN:(t + 1) * F_IN],
                    scalar=1.0, in1=w1s[:],
                    op0=mybir.AluOpType.mult, op1=mybir.AluOpType.mult,
                    accum_out=s1[:],
                )
                ssum = scpool.tile([128, 1], F32, tag="ss")
                nc.vector.tensor_tensor(
                    out=ssum[:], in0=s0[:], in1=s1[:], op=mybir.AluOpType.add
                )
                nc.scalar.activation(
                    out=zsb[:, t:t + 1], in_=ssum[:],
                    func=mybir.ActivationFunctionType.Sigmoid, bias=bls[:, :1],
                )
            nc.sync.dma_start(out=z[:], in_=zsb[:])
    nc.compile()
    return nc


def _run(name, nc, in_maps, trace=True):
    last = None
    for attempt in range(3):
        try:
            res = run_bass_kernel_spmd(
                nc, in_maps, core_ids=list(range(NCORES)), trace=trace
            )
            LAST_EXEC_NS[name] = res.exec_time_ns
            return res.results
        except Exception as e:  # wedged-device retry (clears on re-attempt)
            last = e
            time.sleep(5)
    raise last


def _rep(v, n=128):
    return np.ascontiguousarray(np.broadcast_to(np.asarray(v, np.float32), (n, len(v))))


def _tile_xT(xfull_shards, kc, d_in):
    """list of [NSP, d_in] fp16 per core -> [NCORES, kc, 128, W, 128] fp16
    (partition-major: xT[c,k,p,w,f] = x[w*128+f? no: x^T tiles)."""
    out = np.zeros((NCORES, kc, 128, W, 128), np.float16)
    for c in range(NCORES):
        xt = xfull_shards[c].T  # [d_in, NSP]
        for k in range(kc):
            blk = xt[128 * k:128 * (k + 1)].reshape(128, W, 128)
            out[c, k] = blk
    return out


# ------------------------------------------------------------------- kernel
def kernel(features, edge_index, mask, W1, a_src1, a_dst1, b1, W2, a_src2,
           a_dst2, b2, Wl, bl):
    features = np.asarray(features, np.float32)
    edge_index = np.asarray(edge_index, np.int32)
    mask = np.asarray(mask, np.int32)
    W1, W2, Wl = (np.asarray(a, np.float32) for a in (W1, W2, Wl))
    a_src1, a_dst1, b1 = (np.asarray(a, np.float32) for a in (a_src1, a_dst1, b1))
    a_src2, a_dst2, b2 = (np.asarray(a, np.float32) for a in (a_src2, a_dst2, b2))
    bl = np.asarray(bl, np.float32)

    g = _prep_graph(edge_index)
    T, WT = g["T"], g["WT"]

    key = (T, WT)
    if key not in _PROG_CACHE:
        _PROG_CACHE[key] = dict(
            p1=_build_proj(1, H),
            a1=_build_agg(H, T, WT, relu=True, proj_d2=F_IN),
            a2=_build_agg(F_IN, T, WT, relu=False),
            lk=_build_link((10000 // NCORES + 127) // 128),
        )
    progs = _PROG_CACHE[key]

    # ---- L1: H1 = X @ W1 (sharded, window-permuted rows), es1/ed1
    xsh = []
    for c in range(NCORES):
        xs = np.zeros((NSP, F_IN), np.float16)
        r2n = g["row2node"][c]
        m = r2n >= 0
        xs[m] = features[r2n[m]]
        xsh.append(xs)
    xT1 = _tile_xT(xsh, 1, F_IN)
    W1h = W1.astype(np.float16)
    bex1 = np.concatenate([b1, [0.0, 0.0]]).astype(np.float16)[None, :]
    r1 = _run("p1", progs["p1"], [
        dict(xT=xT1[c], Wm=W1h, asr=_rep(a_src1), adr=_rep(a_dst1), bex=bex1)
        for c in range(NCORES)
    ])
    H1e = _full_from_shards([r1[c]["h16"] for c in range(NCORES)], g, H)
    esed1 = _full_from_shards(
        [r1[c]["esed"].reshape(128, W, 2) for c in range(NCORES)], g, 2)
    es1, ed1 = esed1[:, 0], esed1[:, 1]

    # ---- L2: agg layer 1 (b1 in table rows) fused with proj2 -> h2, es2/ed2
    W2h = W2.astype(np.float16)
    bex2 = np.concatenate([b2, [0.0, 0.0]]).astype(np.float16)[None, :]
    idt16 = np.eye(128, dtype=np.float16)
    ins2 = []
    for c in range(NCORES):
        esx, edx = _expand(es1, ed1, g, c)
        ins2.append(dict(gt=_gt_tiles(H1e, g, c), eqm=g["eq8"][c],
                         esx=esx, edx=edx, Wm2=W2h, asr2=_rep(a_src2),
                         adr2=_rep(a_dst2), bex2=bex2, idt=idt16))
    r2 = _run("a1", progs["a1"], ins2)
    H2e = _full_from_shards([r2[c]["h2o"] for c in range(NCORES)], g, F_IN)
    esed2 = _full_from_shards(
        [r2[c]["esed2"].reshape(128, W, 2) for c in range(NCORES)], g, 2)
    es2, ed2 = esed2[:, 0], esed2[:, 1]

    # ---- L4: aggregate layer 2 -> h2 = agg (b2 already in table rows)
    ins4 = []
    for c in range(NCORES):
        esx, edx = _expand(es2, ed2, g, c)
        ins4.append(dict(gt=_gt_tiles(H2e, g, c), eqm=g["eq8"][c],
                         esx=esx, edx=edx))
    r4 = _run("a2", progs["a2"], ins4)
    H2f = _full_from_shards([r4[c]["ho"] for c in range(NCORES)], g, F_IN)

    # ---- L5: link predictor (host-staged row tiles)
    P = mask.shape[0]
    pc = P // NCORES
    pt = (pc + 127) // 128
    mT = mask.T
    wl0 = _rep(Wl[:F_IN, 0])
    wl1 = _rep(Wl[F_IN:, 0])
    blr = np.full((128, 1), float(bl[0]), np.float32)
    ins5 = []
    for c in range(NCORES):
        m0 = np.zeros((128, pt), np.int64)
        m1 = np.zeros((128, pt), np.int64)
        s = np.arange(pc)
        m0[s % 128, s // 128] = mT[0][c * pc:(c + 1) * pc]
        m1[s % 128, s // 128] = mT[1][c * pc:(c + 1) * pc]
        g0 = H2f[m0].reshape(128, pt * F_IN)
        g1 = H2f[m1].reshape(128, pt * F_IN)
        ins5.append(dict(g0=g0, g1=g1, wl0=wl0, wl1=wl1, blr=blr))
    r5 = _run("lk", progs["lk"], ins5)
    out = np.zeros((P, 1), np.float32)
    for c in range(NCORES):
        s = np.arange(pc)
        out[c * pc:(c + 1) * pc, 0] = r5[c]["z"][s % 128, s // 128]

    tot = sum(v for v in LAST_EXEC_NS.values() if v)
    print(f"kernel launches ns: {LAST_EXEC_NS} total {tot}")
    return out
